# revision 43
# baseline (speedup 1.0000x reference)
"""Trainium2 Bass kernel for nn_DiscretisedBNF (histogram binning MLP).

Math: the reference's per-bin CDF sum telescopes exactly (kl_{k+1} == kr_k
bit-identically, and cdf(kl_0) = cdf(kr_0) = 0 since those bounds are <= -1),
so

    sum_k [cdf(kr_k) - cdf(kl_k)] = cdf(kr_{K-1}) = 0.5*(1 + erf((0.875-mu_x)*inv))

with mu_x = mu/gamma - s*mu_eps, inv = 1/(sigma_x*sqrt(2)), sigma_x =
s*exp(ln_sigma_eps), s = sqrt((1-gamma)/gamma).  Rearranged for the chip,
with every per-column constant folded on the host:

    arg = (psA + M) * e              psA = 2^12 * (h @ W2A')     (PSUM)
    M   = 2^12*(mu*qm + qa + b2A)*EB   (host-precomputed, bf16)
    e   = exp(-2^-12*psB - ln(sqrt2) - 12ln2)                    (= E*2^-12)
    W2A'= W2A * EB,  EB = exp(-b2B)   (b2 of the B half folded as a
                                       multiplicative per-col factor)
    out = 0.5*erf(arg) + 0.5

Precision: both matmuls run in fp8 e4m3 with perf_mode=DoubleRow (2 fp8
weights/cell, ~2x PE rate, half the fp16 DMA bytes).  Scales keep everything
in e4m3's normal range (max +-240): x by 2^4, W1/W2 by 2^8.  The b1 bias and
the t-row of the concat([mu,t]) input are seeded into the mm1 PSUM
accumulation by a single rank-2 matmul per m-chunk (so mm1's streamed
contraction is exactly D=4096 = 16 DoubleRow pairs, and the Lrelu needs no
per-tile bias -> two merged 1024-wide Lrelus).  Epilogue intermediates are
bf16.  Measured end-to-end rel err: 1.0e-2 vs the 2e-2 gate.

Sharding: pure data parallel - batch dim (2048) split 256 rows per core;
weights replicated.  DoubleRow wants k-chunk PAIRS interleaved on the same
128 partitions (AP [128, 2, free]); the host packs accordingly.

Scheduling notes (from HW traces):
- Every bulk DMA is one contiguous multi-KB run per partition (flat 2D dram
  tensors + flat tiles, matmul views via AP rearrange): the DGE generates
  ~80 descriptors/us per queue, so small-descriptor patterns cap a ring at
  ~100-160 GB/s while 4-8KB runs reach full HBM bandwidth.
- The HAM clock gate needs a few us of PE-busy to open (warm-up burst) and
  closes again on PE idle >~2-3us; rank-2 seeds don't count as busy, so a
  few more warm-up matmuls follow them.
- The ACT engine reloads its function table on every Exp<->Erf switch
  (1.3-1.5us), so Erfs are batched ([4,2,2]) behind the Exps whose PSUM
  releases gate the next j-group's matmuls, with dummy activations
  preloading the Lrelu/Exp tables during idle windows.
- PSUM is managed as [128,1024] two-bank tiles: mm1 uses 2 (4 m-chunks
  each), mm2 one psA + one psB per j, so the epilogue runs 1024-wide ops.
- The last j computes its B half first and splits the epilogue chain into
  512-wide halves so the tail after the final matmul is short.
"""

import numpy as np
import ml_dtypes
from contextlib import ExitStack

import concourse.bass as bass
import concourse.mybir as mybir
from concourse.tile import TileContext
from concourse.tile_rust import add_dep_helper
from concourse.bass_utils import run_bass_kernel_spmd

B, D, H = 2048, 4096, 1024
NCORES = 8
BS = B // NCORES            # 256 batch rows per core
KP1 = 16                    # mm1 streamed contract pairs: 16*256 = 4096 = D
KC2 = H // 128              # 8 contract chunks for matmul2
KP2 = KC2 // 2              # 4 DoubleRow pairs
NJ = D // 512               # 8 output column groups of 512
LEAKY_SLOPE = 0.01
LN_SQRT2 = 0.34657359027997264
LN2 = 0.6931471805599453
SX = 2.0**4                 # x fp8 scale
SW = 2.0**8                 # W1/W2 fp8 scale

F8 = mybir.dt.float8e4
F16 = mybir.dt.float16
BF16 = mybir.dt.bfloat16
F32 = mybir.dt.float32
AF = mybir.ActivationFunctionType
OP = mybir.AluOpType
DR = mybir.MatmulPerfMode.DoubleRow

NPF8 = ml_dtypes.float8_e4m3
NPBF16 = ml_dtypes.bfloat16


def split_multi_waits(nc):
    """This container's walrus accepts at most ONE sync-wait per instruction
    (setupSyncWait: 'Too many sync wait commands').  Split any instruction
    carrying N>1 waits into N-1 single-wait NoOps on the same engine placed
    immediately before it."""
    cnt = 0
    sync_info_cls = None
    for f in nc.m.functions:
        for bb in f.blocks:
            out = []
            changed = False
            for inst in bb.instructions:
                si = inst.sync_info
                waits = list(si.on_wait) if si and si.on_wait else []
                if len(waits) > 1:
                    if sync_info_cls is None:
                        sync_info_cls = type(si)
                    for w in waits[:-1]:
                        nop = mybir.InstNoOp(name=f"waitsplit_{cnt}", ins=[], outs=[])
                        cnt += 1
                        nop.engine = inst.engine
                        nop.sync_info = sync_info_cls(on_wait=[w], on_update=[])
                        out.append(nop)
                    si.on_wait = waits[-1:]
                    changed = True
                out.append(inst)
            if changed:
                bb.instructions = out
    return cnt


def _lean_drain_and_barrier(self, tick_clock, wait_clock):
    """Replacement for TileContext._drain_and_barrier: drain + ONE barrier,
    skipping the ~7us semaphore-clear butterfly.  The Bass preamble re-clears
    every kernel semaphore at the start of each execution, and no sibling
    TileContext follows this one, so the tail clear is redundant.  The
    multi-wait drain is split later by split_multi_waits."""
    import concourse.tile as tile_mod

    nc = self.nc
    drain_inst = nc.sync.drain()
    wait_clock.add_sem_waits(
        drain_inst.ins, tile_mod.ScopedClock({None: tick_clock.global_clock})
    )
    # No all_engine_barrier: the SP drain above waits on every semaphore's
    # final tick (all engines' last work and all DMA completions), so SP
    # retires last and execution end implies everything finished.
    popped = nc._tile_sem_poison_stack.pop()
    assert popped is self._sem_poison


def _build():
    nc = bass.Bass()
    orig_drain = TileContext._drain_and_barrier
    TileContext._drain_and_barrier = _lean_drain_and_barrier
    try:
        _build_body(nc)
    finally:
        TileContext._drain_and_barrier = orig_drain

    split_multi_waits(nc)
    return nc


def _build_body(nc):
    # All bulk tensors are FLAT per-partition so every DMA is one contiguous
    # multi-KB run per partition: the DGE generates descriptors at a fixed
    # ~80/us per queue, so sub-2KB descriptors cap a ring at ~160 GB/s
    # (measured: W1's 4D pattern starved mm1 at ~70-125 GB/s).
    xT = nc.dram_tensor("xT", [128, KP1 * 2 * BS], F8, kind="ExternalInput")
    w1 = nc.dram_tensor("w1", [128, KP1 * 2 * H], F8, kind="ExternalInput")
    w2 = nc.dram_tensor("w2", [128, NJ, KC2, 2, 512], F8, kind="ExternalInput")
    # rank-2 seed operands: row0 = (b1*2^12, ones), row1 = (W1[4096]*2^8, t*2^4)
    seedw = nc.dram_tensor("seedw", [2, H], F8, kind="ExternalInput")
    seedx = nc.dram_tensor("seedx", [2, BS], F8, kind="ExternalInput")
    # mun/out packed [p, j, h, col] -> flat [128, 8192]; host un/packs
    mun = nc.dram_tensor("mun", [128, NJ * 1024], BF16, kind="ExternalInput")
    outd = nc.dram_tensor("out", [128, NJ * 1024], F16, kind="ExternalOutput")

    with TileContext(nc) as tc, ExitStack() as ctx:
        const = ctx.enter_context(tc.tile_pool(name="const", bufs=1))
        xpool = ctx.enter_context(tc.tile_pool(name="xpool", bufs=1))
        w1pool = ctx.enter_context(tc.tile_pool(name="w1pool", bufs=4))
        hpool = ctx.enter_context(tc.tile_pool(name="hpool", bufs=1))
        w2pool = ctx.enter_context(tc.tile_pool(name="w2pool", bufs=6))
        eppool = ctx.enter_context(tc.tile_pool(name="eppool", bufs=4))
        outpool = ctx.enter_context(tc.tile_pool(name="outpool", bufs=3))
        pspool = ctx.enter_context(tc.tile_pool(name="pspool", bufs=4, space="PSUM"))

        # --- constants ---
        ones_row = const.tile([128, BS], F16, name="ones_row")
        nc.vector.memset(ones_row[:], 1.0)
        ones128 = const.tile([128, 128], F16, name="ones128")
        nc.vector.memset(ones128[:], 1.0)
        nln2_sb = const.tile([128, 1], F32, name="nln2_sb")
        nc.vector.memset(nln2_sb[:], -(LN_SQRT2 + 12.0 * LN2))
        scratch = const.tile([128, 1], F32, name="scratch")

        # preload the Lrelu ACT table (1.5us load, hidden under mm1)
        nc.scalar.activation(scratch[:], nln2_sb[:], AF.Lrelu, alpha=LEAKY_SLOPE)

        # short PE warm-up: dependency-free full-rank matmuls open the HAM
        # clock gate (needs ~3.5us of PE-busy; rank-2 seeds don't count)
        ps_warm = pspool.tile([128, 1024], F32, tag="ps", name="ps_warm")

        def warmup(n):
            for _ in range(n):
                nc.tensor.matmul(
                    ps_warm[:, :BS], ones128[:], ones_row[:], start=True, stop=True
                )

        warmup(10)

        # tiny rank-2 seed operand loads first on the SWDGE ring
        seedw_sb = const.tile([2, H], F8, name="seedw_sb")
        nc.gpsimd.dma_start(out=seedw_sb[:], in_=seedw[:])
        seedx_sb = const.tile([2, BS], F8, name="seedx_sb")
        nc.gpsimd.dma_start(out=seedx_sb[:], in_=seedx[:])

        # --- x^T resident, pair-packed; Scalar HWDGE ring (the SWDGE ring
        # ramps far too slowly at kernel start), one flat tile, split so
        # mm1's first pairs don't wait for the whole 1 MB.
        XT_PARTS = [4, 6, 6]  # pairs per part; front-load small
        xt_flat = xpool.tile([128, KP1 * 2 * BS], F8, name="xt_flat")
        q0 = 0
        for npair in XT_PARTS:
            nc.scalar.dma_start(
                out=xt_flat[:, q0 * 512 : (q0 + npair) * 512],
                in_=xT[:, q0 * 512 : (q0 + npair) * 512],
            )
            q0 += npair
        assert q0 == KP1

        def xt_pair(q):
            return xt_flat[:, q * 512 : (q + 1) * 512].rearrange(
                "p (r b) -> p r b", r=2
            )

        # --- matmul1: h^T = W1^T @ x^T, H on partitions, fp8 DoubleRow.
        # PSUM: two [128,1024] double-bank tiles, 4 m-chunks (256 cols) each.
        ps1_t = [
            pspool.tile([128, 1024], F32, tag="ps", name=f"ps1_t{i}")
            for i in range(2)
        ]

        def ps1(m):
            return ps1_t[m // 4][:, (m % 4) * BS : (m % 4 + 1) * BS]

        # rank-2 seeds (b1 bias + t-row contribution) start the mm1
        # accumulation; a few more warm-up matmuls after them bridge the
        # HAM-invisible window (rank-2 work doesn't count as PE-busy).
        for m in range(KC2):
            nc.tensor.matmul(
                ps1(m),
                seedw_sb[:, m * 128 : (m + 1) * 128],
                seedx_sb[:],
                start=True,
                stop=False,
            )
        warmup(4)

        W1_PARTS = [2, 2, 4, 4, 4]  # pairs per group, Sync ring, flat tiles
        PW = 2 * H  # elements per pair per partition
        mm1_last = {}
        q = 0
        for g, npair in enumerate(W1_PARTS):
            w1g = w1pool.tile(
                [128, max(W1_PARTS) * PW], F8, tag="w1t", name=f"w1g{g}"
            )
            nc.sync.dma_start(
                out=w1g[:, : npair * PW],
                in_=w1[:, q * PW : (q + npair) * PW],
            )
            for i in range(npair):
                rhs = xt_pair(q)
                w1p = w1g[:, i * PW : (i + 1) * PW].rearrange(
                    "p (r h) -> p r h", r=2
                )
                for m in range(KC2):
                    mm = nc.tensor.matmul(
                        ps1(m),
                        w1p[:, :, m * 128 : (m + 1) * 128],
                        rhs,
                        start=False,
                        stop=(q == KP1 - 1),
                        perf_mode=DR,
                    )
                mm1_last[q] = mm
                q += 1
        assert q == KP1

        # h fp8 at 2^4 scale, all 8 chunks in one [128, 8, 256] tile;
        # Lrelu(2^-8 * psum) = 2^4 * Lrelu(xW1 + b1): bias was seeded, so
        # TWO merged 1024-wide Lrelus cover all 8 chunks.
        h_all = hpool.tile([128, KC2, BS], F8, name="h_all")
        for i in range(2):
            nc.scalar.activation(
                h_all[:, 4 * i : 4 * (i + 1), :],
                ps1_t[i][:],
                AF.Lrelu,
                scale=2.0**-8,
                alpha=LEAKY_SLOPE,
            )
        # preload the Exp table while mm2's first matmuls run
        nc.scalar.activation(scratch[:], nln2_sb[:], AF.Exp)

        # --- matmul2 + fused epilogue ---
        w2ts, mu_js = {}, {}
        for j in range(NJ):
            w2t = w2pool.tile([128, KC2, 2, 512], F8, tag="w2", name=f"w2t{j}")
            pace = {0: 7, 1: 10, 2: 13, 3: 15}.get(j)
            dma = nc.gpsimd.dma_start(out=w2t[:], in_=w2[:, j])
            if pace is not None:
                add_dep_helper(dma.ins, mm1_last[pace].ins, True, "pace w2")
            w2ts[j] = w2t
            mu_j = eppool.tile([128, 1024], BF16, tag="mu", name=f"mu{j}")
            dma = nc.scalar.dma_start(
                out=mu_j[:], in_=mun[:, j * 1024 : (j + 1) * 1024]
            )
            if j == 0:
                add_dep_helper(dma.ins, mm1_last[13].ins, True, "pace mu")
            mu_js[j] = mu_j

        def emit_matmuls(j, b_first):
            w2t = w2ts[j]
            psA = pspool.tile([128, 1024], F32, tag="ps", name=f"psA{j}")
            psB = pspool.tile([128, 1024], F32, tag="ps", name=f"psB{j}")
            dst = {0: psA, 1: psB}
            for h in ((1, 0) if b_first else (0, 1)):
                for qq in range(KP2):
                    for bh in range(2):
                        nc.tensor.matmul(
                            dst[h][:, bh * 512 : (bh + 1) * 512],
                            h_all[:, 2 * qq : 2 * qq + 2, bh * 128 : (bh + 1) * 128],
                            w2t[:, 2 * qq : 2 * qq + 2, h, :],
                            start=(qq == 0),
                            stop=(qq == KP2 - 1),
                            perf_mode=DR,
                        )
            return psA, psB

        g2s = {}
        pend_erf = []

        def flush_erfs(js):
            for j in js:
                o2 = outpool.tile([128, 1024], F16, tag="o", name=f"O{j}")
                if j == NJ - 1:
                    # split the last chain across ACT/DVE/GpSimd halves so
                    # the tail after the final matmul is ~half as deep
                    for bh in range(2):
                        sl = slice(bh * 512, (bh + 1) * 512)
                        r2 = eppool.tile([128, 512], BF16, tag="R", name=f"R{j}_{bh}")
                        nc.scalar.activation(r2[:], g2s[j][:, sl], AF.Erf)
                        eng = nc.gpsimd if bh == 0 else nc.vector
                        eng.tensor_scalar(o2[:, sl], r2[:], 0.5, 0.5, OP.mult, OP.add)
                        nc.sync.dma_start(
                            out=outd[:, j * 1024 + bh * 512 : j * 1024 + (bh + 1) * 512],
                            in_=o2[:, sl],
                        )
                else:
                    r2 = eppool.tile([128, 1024], BF16, tag="Rw", name=f"R{j}")
                    nc.scalar.activation(r2[:], g2s[j][:], AF.Erf)
                    nc.gpsimd.tensor_scalar(o2[:], r2[:], 0.5, 0.5, OP.mult, OP.add)
                    nc.sync.dma_start(
                        out=outd[:, j * 1024 : (j + 1) * 1024], in_=o2[:]
                    )

        # Flush AFTER j4/j6 so each Erf batch sits behind the Exp whose PSUM
        # release the j+2 matmuls need — an Erf batch queued before that Exp
        # stalls the PE on psB slots.
        ERF_FLUSH_AFTER = {4, 6}  # Erf batches [4,2,2]

        for j in range(NJ):
            last = j == NJ - 1
            psA, psB = emit_matmuls(j, b_first=last)
            s2 = eppool.tile([128, 1024], BF16, tag="S", name=f"S{j}")
            g2 = eppool.tile([128, 1024], BF16, tag="G", name=f"G{j}")
            e2 = eppool.tile([128, 1024], BF16, tag="E", name=f"E{j}")
            if last:
                # split the chain into 512-wide halves on the DVE (GpSimd
                # cannot read PSUM and is ~2x slower on tensor_tensor) so
                # the bh=0 half's Erf/out can start one op earlier.
                for bh in range(2):
                    sl = slice(bh * 512, (bh + 1) * 512)
                    nc.vector.tensor_tensor(
                        s2[:, sl], psA[:, sl], mu_js[j][:, sl], OP.add
                    )
                nc.scalar.activation(
                    e2[:], psB[:], AF.Exp, bias=nln2_sb[:], scale=-(2.0**-12)
                )
                for bh in range(2):
                    sl = slice(bh * 512, (bh + 1) * 512)
                    nc.vector.tensor_tensor(g2[:, sl], s2[:, sl], e2[:, sl], OP.mult)
            else:
                nc.vector.tensor_tensor(s2[:], psA[:], mu_js[j][:], OP.add)
                nc.scalar.activation(
                    e2[:], psB[:], AF.Exp, bias=nln2_sb[:], scale=-(2.0**-12)
                )
                nc.vector.tensor_tensor(g2[:], s2[:], e2[:], OP.mult)
            g2s[j] = g2
            pend_erf.append(j)
            if j in ERF_FLUSH_AFTER:
                flush_erfs(pend_erf[:-1])
                del pend_erf[:-1]
        flush_erfs(pend_erf)


_NC = None
_last_in_maps = None


def kernel(mu, t, gamma, W1, b1, W2, b2):
    global _NC
    if _NC is None:
        _NC = _build()
    nc = _NC

    f16 = np.float16
    f32 = np.float32

    def q8(a, scale):
        return np.clip(np.asarray(a, f32) * scale, -240.0, 240.0).astype(NPF8)

    # x^T = mu^T * 2^4 fp8, pair-packed [128, (q, r, b)] flat; the t column
    # is a rank-2 seed (with b1)
    Xt = q8(mu, SX).T                      # (D, B)
    w1_np = np.ascontiguousarray(
        q8(W1[: D], SW).reshape(KP1, 2, 128, H).transpose(2, 0, 1, 3)
    ).reshape(128, KP1 * 2 * H)
    seedw_np = np.stack([q8(b1, SX * SW), q8(W1[D], SW)])  # [2, H]

    b2_64 = np.asarray(b2, np.float64)
    b2A, b2B = b2_64[:D], b2_64[D:]
    EB = np.exp(-b2B)  # fold b2 of the B half as a per-col factor on W2A
    W2f = np.asarray(W2, f32).astype(np.float64)
    W2q = np.concatenate([W2f[:, :D] * EB[None, :], W2f[:, D:]], axis=1)
    # W2 pack [p, j, k, half, col] = W2q[k*128+p, half*D + j*512 + col]
    w2_np = np.ascontiguousarray(
        q8(W2q, SW).reshape(KC2, 128, 2, NJ, 512).transpose(1, 3, 0, 2, 4)
    )

    g64 = np.asarray(gamma, dtype=np.float64)[:, 0]
    s64 = np.sqrt((1.0 - g64) / g64)
    qm_t = -1.0 / (g64 * s64)
    qa_t = 0.875 / s64
    # M = 2^12 * (mu*qm + qa + b2A) * EB, bf16  (absmax ~4e5, well in range)
    mun2 = (
        (
            np.asarray(mu, np.float64) * qm_t[:, None]
            + qa_t[:, None]
            + b2A[None, :]
        )
        * EB[None, :]
        * (SX * SW)
    ).astype(NPBF16)
    t8 = q8(t, SX)  # (B, 1)

    in_maps = []
    for c in range(NCORES):
        sl = slice(c * BS, (c + 1) * BS)
        in_maps.append(
            {
                "xT": np.ascontiguousarray(
                    Xt[:, sl].reshape(KP1, 2, 128, BS).transpose(2, 0, 1, 3)
                ).reshape(128, KP1 * 2 * BS),
                "w1": w1_np,
                "w2": w2_np,
                "seedw": seedw_np,
                "seedx": np.stack(
                    [np.ones(BS, dtype=NPF8), t8[sl, 0].astype(NPF8)]
                ),
                # [p, j, h, c] = M[h*128+p, j*512+c], flat [128, 8192]
                "mun": np.ascontiguousarray(
                    mun2[sl].reshape(2, 128, NJ, 512).transpose(1, 2, 0, 3)
                ).reshape(128, NJ * 1024),
            }
        )

    global _last_in_maps
    _last_in_maps = in_maps

    res = run_bass_kernel_spmd(nc, in_maps, core_ids=list(range(NCORES)))
    # out dram is [p, (j, h, c)]; unpack to [b, d] = [h*128+p, j*512+c]
    return np.concatenate(
        [
            r["out"]
            .reshape(128, NJ, 2, 512)
            .transpose(2, 0, 1, 3)
            .reshape(BS, D)
            .astype(np.float32)
            for r in res.results
        ],
        axis=0,
    )


# revision 44
# speedup vs baseline: 1.0944x; 1.0944x over previous
"""Trainium2 Bass kernel for nn_DiscretisedBNF (histogram binning MLP).

Math: the reference's per-bin CDF sum telescopes exactly (kl_{k+1} == kr_k
bit-identically, and cdf(kl_0) = cdf(kr_0) = 0 since those bounds are <= -1),
so

    sum_k [cdf(kr_k) - cdf(kl_k)] = cdf(kr_{K-1}) = 0.5*(1 + erf((0.875-mu_x)*inv))

with mu_x = mu/gamma - s*mu_eps, inv = 1/(sigma_x*sqrt(2)), sigma_x =
s*exp(ln_sigma_eps), s = sqrt((1-gamma)/gamma).  Rearranged for the chip,
with every per-column constant folded on the host:

    arg = (psA + M) * e              psA = 2^12 * (h @ W2A')     (PSUM)
    M   = 2^12*(mu*qm + qa + b2A)*EB   (host-precomputed, bf16)
    e   = exp(-2^-12*psB - ln(sqrt2) - 12ln2)                    (= E*2^-12)
    W2A'= W2A * EB,  EB = exp(-b2B)   (b2 of the B half folded as a
                                       multiplicative per-col factor)
    out = 0.5*erf(arg) + 0.5

Precision: both matmuls run in fp8 e4m3 with perf_mode=DoubleRow (2 fp8
weights/cell, ~2x PE rate, half the fp16 DMA bytes).  Scales keep everything
in e4m3's normal range (max +-240): x by 2^4, W1/W2 by 2^8.  The b1 bias and
the t-row of the concat([mu,t]) input are seeded into the mm1 PSUM
accumulation by a single rank-2 matmul per m-chunk (so mm1's streamed
contraction is exactly D=4096 = 16 DoubleRow pairs, and the Lrelu needs no
per-tile bias -> two merged 1024-wide Lrelus).  Epilogue intermediates are
bf16.  Measured end-to-end rel err: 1.0e-2 vs the 2e-2 gate.

Sharding: pure data parallel - batch dim (2048) split 256 rows per core;
weights replicated.  DoubleRow wants k-chunk PAIRS interleaved on the same
128 partitions (AP [128, 2, free]); the host packs accordingly.

Scheduling notes (from HW traces):
- Every bulk DMA is one contiguous multi-KB run per partition (flat 2D dram
  tensors + flat tiles, matmul views via AP rearrange): the DGE generates
  ~80 descriptors/us per queue, so small-descriptor patterns cap a ring at
  ~100-160 GB/s while 4-8KB runs reach full HBM bandwidth.
- The HAM clock gate needs a few us of PE-busy to open (warm-up burst) and
  closes again on PE idle >~2-3us; rank-2 seeds don't count as busy, so a
  few more warm-up matmuls follow them.
- The ACT engine reloads its function table on every Exp<->Erf switch
  (1.3-1.5us), so Erfs are batched ([4,2,2]) behind the Exps whose PSUM
  releases gate the next j-group's matmuls, with dummy activations
  preloading the Lrelu/Exp tables during idle windows.
- PSUM is managed as [128,1024] two-bank tiles: mm1 uses 2 (4 m-chunks
  each), mm2 one psA + one psB per j, so the epilogue runs 1024-wide ops.
- The last j computes its B half first and splits the epilogue chain into
  512-wide halves so the tail after the final matmul is short.
"""

import numpy as np
import ml_dtypes
from contextlib import ExitStack

import concourse.bass as bass
import concourse.mybir as mybir
from concourse.tile import TileContext
from concourse.tile_rust import add_dep_helper
from concourse.bass_utils import run_bass_kernel_spmd

B, D, H = 2048, 4096, 1024
NCORES = 8
BS = B // NCORES            # 256 batch rows per core
KP1 = 16                    # mm1 streamed contract pairs: 16*256 = 4096 = D
KC2 = H // 128              # 8 contract chunks for matmul2
KP2 = KC2 // 2              # 4 DoubleRow pairs
NJ = D // 512               # 8 output column groups of 512
LEAKY_SLOPE = 0.01
LN_SQRT2 = 0.34657359027997264
LN2 = 0.6931471805599453
SX = 2.0**4                 # x fp8 scale
SW = 2.0**8                 # W1/W2 fp8 scale

F8 = mybir.dt.float8e4
F16 = mybir.dt.float16
BF16 = mybir.dt.bfloat16
F32 = mybir.dt.float32
AF = mybir.ActivationFunctionType
OP = mybir.AluOpType
DR = mybir.MatmulPerfMode.DoubleRow

NPF8 = ml_dtypes.float8_e4m3
NPBF16 = ml_dtypes.bfloat16


def split_multi_waits(nc):
    """This container's walrus accepts at most ONE sync-wait per instruction
    (setupSyncWait: 'Too many sync wait commands').  Split any instruction
    carrying N>1 waits into N-1 single-wait NoOps on the same engine placed
    immediately before it."""
    cnt = 0
    sync_info_cls = None
    for f in nc.m.functions:
        for bb in f.blocks:
            out = []
            changed = False
            for inst in bb.instructions:
                si = inst.sync_info
                waits = list(si.on_wait) if si and si.on_wait else []
                if len(waits) > 1:
                    if sync_info_cls is None:
                        sync_info_cls = type(si)
                    for w in waits[:-1]:
                        nop = mybir.InstNoOp(name=f"waitsplit_{cnt}", ins=[], outs=[])
                        cnt += 1
                        nop.engine = inst.engine
                        nop.sync_info = sync_info_cls(on_wait=[w], on_update=[])
                        out.append(nop)
                    si.on_wait = waits[-1:]
                    changed = True
                out.append(inst)
            if changed:
                bb.instructions = out
    return cnt


def _lean_drain_and_barrier(self, tick_clock, wait_clock):
    """Replacement for TileContext._drain_and_barrier: drain + ONE barrier,
    skipping the ~7us semaphore-clear butterfly.  The Bass preamble re-clears
    every kernel semaphore at the start of each execution, and no sibling
    TileContext follows this one, so the tail clear is redundant.  The
    multi-wait drain is split later by split_multi_waits."""
    import concourse.tile as tile_mod

    nc = self.nc
    drain_inst = nc.sync.drain()
    wait_clock.add_sem_waits(
        drain_inst.ins, tile_mod.ScopedClock({None: tick_clock.global_clock})
    )
    # No all_engine_barrier: the SP drain above waits on every semaphore's
    # final tick (all engines' last work and all DMA completions), so SP
    # retires last and execution end implies everything finished.
    popped = nc._tile_sem_poison_stack.pop()
    assert popped is self._sem_poison


def _build():
    nc = bass.Bass()
    orig_drain = TileContext._drain_and_barrier
    TileContext._drain_and_barrier = _lean_drain_and_barrier
    try:
        _build_body(nc)
    finally:
        TileContext._drain_and_barrier = orig_drain

    split_multi_waits(nc)
    return nc


def _build_body(nc):
    # All bulk tensors are FLAT per-partition so every DMA is one contiguous
    # multi-KB run per partition: the DGE generates descriptors at a fixed
    # ~80/us per queue, so sub-2KB descriptors cap a ring at ~160 GB/s
    # (measured: W1's 4D pattern starved mm1 at ~70-125 GB/s).
    xT = nc.dram_tensor("xT", [128, KP1 * 2 * BS], F8, kind="ExternalInput")
    w1 = nc.dram_tensor("w1", [128, KP1 * 2 * H], F8, kind="ExternalInput")
    w2 = nc.dram_tensor("w2", [128, NJ, KC2, 2, 512], F8, kind="ExternalInput")
    # rank-2 seed operands: row0 = (b1*2^12, ones), row1 = (W1[4096]*2^8, t*2^4)
    seedw = nc.dram_tensor("seedw", [2, H], F8, kind="ExternalInput")
    seedx = nc.dram_tensor("seedx", [2, BS], F8, kind="ExternalInput")
    # mun/out packed [p, j, h, col] -> flat [128, 8192]; host un/packs
    mun = nc.dram_tensor("mun", [128, NJ * 1024], BF16, kind="ExternalInput")
    outd = nc.dram_tensor("out", [128, NJ * 1024], F16, kind="ExternalOutput")

    with TileContext(nc) as tc, ExitStack() as ctx:
        const = ctx.enter_context(tc.tile_pool(name="const", bufs=1))
        xpool = ctx.enter_context(tc.tile_pool(name="xpool", bufs=1))
        w1pool = ctx.enter_context(tc.tile_pool(name="w1pool", bufs=4))
        hpool = ctx.enter_context(tc.tile_pool(name="hpool", bufs=1))
        w2pool = ctx.enter_context(tc.tile_pool(name="w2pool", bufs=6))
        eppool = ctx.enter_context(tc.tile_pool(name="eppool", bufs=4))
        outpool = ctx.enter_context(tc.tile_pool(name="outpool", bufs=3))
        pspool = ctx.enter_context(tc.tile_pool(name="pspool", bufs=4, space="PSUM"))

        # --- constants ---
        ones_row = const.tile([128, BS], F16, name="ones_row")
        nc.vector.memset(ones_row[:], 1.0)
        ones128 = const.tile([128, 128], F16, name="ones128")
        nc.vector.memset(ones128[:], 1.0)
        nln2_sb = const.tile([128, 1], F32, name="nln2_sb")
        nc.vector.memset(nln2_sb[:], -(LN_SQRT2 + 12.0 * LN2))
        scratch = const.tile([128, 1], F32, name="scratch")


        # short PE warm-up: dependency-free full-rank matmuls open the HAM
        # clock gate (needs ~3.5us of PE-busy; rank-2 seeds don't count)
        ps_warm = pspool.tile([128, 1024], F32, tag="ps", name="ps_warm")

        def warmup(n):
            for _ in range(n):
                nc.tensor.matmul(
                    ps_warm[:, :BS], ones128[:], ones_row[:], start=True, stop=True
                )

        warmup(10)

        # tiny rank-2 seed operand loads FIRST on the Scalar HWDGE ring
        # (the SWDGE ring takes ~3.5us to deliver its first byte, which
        # stalled the PE between warm-up and mm1 and jittered the HAM boost)
        seedw_sb = const.tile([2, H], F8, name="seedw_sb")
        nc.scalar.dma_start(out=seedw_sb[:], in_=seedw[:])
        seedx_sb = const.tile([2, BS], F8, name="seedx_sb")
        nc.scalar.dma_start(out=seedx_sb[:], in_=seedx[:])

        # --- x^T resident, pair-packed; Scalar HWDGE ring (the SWDGE ring
        # ramps far too slowly at kernel start), one flat tile, split so
        # mm1's first pairs don't wait for the whole 1 MB.
        XT_PARTS = [4, 6, 6]  # pairs per part; front-load small
        xt_flat = xpool.tile([128, KP1 * 2 * BS], F8, name="xt_flat")
        q0 = 0
        for npair in XT_PARTS:
            nc.scalar.dma_start(
                out=xt_flat[:, q0 * 512 : (q0 + npair) * 512],
                in_=xT[:, q0 * 512 : (q0 + npair) * 512],
            )
            q0 += npair
        assert q0 == KP1

        # preload the Lrelu ACT table (1.5us load, hidden under mm1)
        nc.scalar.activation(scratch[:], nln2_sb[:], AF.Lrelu, alpha=LEAKY_SLOPE)

        def xt_pair(q):
            return xt_flat[:, q * 512 : (q + 1) * 512].rearrange(
                "p (r b) -> p r b", r=2
            )

        # --- matmul1: h^T = W1^T @ x^T, H on partitions, fp8 DoubleRow.
        # PSUM: two [128,1024] double-bank tiles, 4 m-chunks (256 cols) each.
        ps1_t = [
            pspool.tile([128, 1024], F32, tag="ps", name=f"ps1_t{i}")
            for i in range(2)
        ]

        def ps1(m):
            return ps1_t[m // 4][:, (m % 4) * BS : (m % 4 + 1) * BS]

        # rank-2 seeds (b1 bias + t-row contribution) start the mm1
        # accumulation; a few more warm-up matmuls after them bridge the
        # HAM-invisible window (rank-2 work doesn't count as PE-busy).
        for m in range(KC2):
            nc.tensor.matmul(
                ps1(m),
                seedw_sb[:, m * 128 : (m + 1) * 128],
                seedx_sb[:],
                start=True,
                stop=False,
            )
        warmup(4)

        W1_PARTS = [2, 2, 4, 4, 4]  # pairs per group, Sync ring, flat tiles
        PW = 2 * H  # elements per pair per partition
        mm1_last = {}
        q = 0
        for g, npair in enumerate(W1_PARTS):
            w1g = w1pool.tile(
                [128, max(W1_PARTS) * PW], F8, tag="w1t", name=f"w1g{g}"
            )
            nc.sync.dma_start(
                out=w1g[:, : npair * PW],
                in_=w1[:, q * PW : (q + npair) * PW],
            )
            for i in range(npair):
                rhs = xt_pair(q)
                w1p = w1g[:, i * PW : (i + 1) * PW].rearrange(
                    "p (r h) -> p r h", r=2
                )
                for m in range(KC2):
                    mm = nc.tensor.matmul(
                        ps1(m),
                        w1p[:, :, m * 128 : (m + 1) * 128],
                        rhs,
                        start=False,
                        stop=(q == KP1 - 1),
                        perf_mode=DR,
                    )
                mm1_last[q] = mm
                q += 1
        assert q == KP1

        # h fp8 at 2^4 scale, all 8 chunks in one [128, 8, 256] tile;
        # Lrelu(2^-8 * psum) = 2^4 * Lrelu(xW1 + b1): bias was seeded, so
        # TWO merged 1024-wide Lrelus cover all 8 chunks.
        h_all = hpool.tile([128, KC2, BS], F8, name="h_all")
        for i in range(2):
            nc.scalar.activation(
                h_all[:, 4 * i : 4 * (i + 1), :],
                ps1_t[i][:],
                AF.Lrelu,
                scale=2.0**-8,
                alpha=LEAKY_SLOPE,
            )
        # preload the Exp table while mm2's first matmuls run
        nc.scalar.activation(scratch[:], nln2_sb[:], AF.Exp)

        # --- matmul2 + fused epilogue ---
        w2ts, mu_js = {}, {}
        for j in range(NJ):
            w2t = w2pool.tile([128, KC2, 2, 512], F8, tag="w2", name=f"w2t{j}")
            pace = {0: 7, 1: 10, 2: 13, 3: 15}.get(j)
            dma = nc.gpsimd.dma_start(out=w2t[:], in_=w2[:, j])
            if pace is not None:
                add_dep_helper(dma.ins, mm1_last[pace].ins, True, "pace w2")
            w2ts[j] = w2t
            mu_j = eppool.tile([128, 1024], BF16, tag="mu", name=f"mu{j}")
            dma = nc.scalar.dma_start(
                out=mu_j[:], in_=mun[:, j * 1024 : (j + 1) * 1024]
            )
            if j == 0:
                add_dep_helper(dma.ins, mm1_last[13].ins, True, "pace mu")
            mu_js[j] = mu_j

        def emit_matmuls(j, b_first):
            w2t = w2ts[j]
            psA = pspool.tile([128, 1024], F32, tag="ps", name=f"psA{j}")
            psB = pspool.tile([128, 1024], F32, tag="ps", name=f"psB{j}")
            dst = {0: psA, 1: psB}
            for h in ((1, 0) if b_first else (0, 1)):
                for qq in range(KP2):
                    for bh in range(2):
                        nc.tensor.matmul(
                            dst[h][:, bh * 512 : (bh + 1) * 512],
                            h_all[:, 2 * qq : 2 * qq + 2, bh * 128 : (bh + 1) * 128],
                            w2t[:, 2 * qq : 2 * qq + 2, h, :],
                            start=(qq == 0),
                            stop=(qq == KP2 - 1),
                            perf_mode=DR,
                        )
            return psA, psB

        g2s = {}
        pend_erf = []

        def flush_erfs(js):
            for j in js:
                o2 = outpool.tile([128, 1024], F16, tag="o", name=f"O{j}")
                if j == NJ - 1:
                    # split the last chain across ACT/DVE/GpSimd halves so
                    # the tail after the final matmul is ~half as deep
                    for bh in range(2):
                        sl = slice(bh * 512, (bh + 1) * 512)
                        r2 = eppool.tile([128, 512], BF16, tag="R", name=f"R{j}_{bh}")
                        nc.scalar.activation(r2[:], g2s[j][:, sl], AF.Erf)
                        eng = nc.gpsimd if bh == 0 else nc.vector
                        eng.tensor_scalar(o2[:, sl], r2[:], 0.5, 0.5, OP.mult, OP.add)
                        nc.sync.dma_start(
                            out=outd[:, j * 1024 + bh * 512 : j * 1024 + (bh + 1) * 512],
                            in_=o2[:, sl],
                        )
                else:
                    r2 = eppool.tile([128, 1024], BF16, tag="Rw", name=f"R{j}")
                    nc.scalar.activation(r2[:], g2s[j][:], AF.Erf)
                    nc.gpsimd.tensor_scalar(o2[:], r2[:], 0.5, 0.5, OP.mult, OP.add)
                    nc.sync.dma_start(
                        out=outd[:, j * 1024 : (j + 1) * 1024], in_=o2[:]
                    )

        # Flush AFTER j4/j6 so each Erf batch sits behind the Exp whose PSUM
        # release the j+2 matmuls need — an Erf batch queued before that Exp
        # stalls the PE on psB slots.
        ERF_FLUSH_AFTER = {4, 6}  # Erf batches [4,2,2]

        for j in range(NJ):
            last = j == NJ - 1
            psA, psB = emit_matmuls(j, b_first=last)
            s2 = eppool.tile([128, 1024], BF16, tag="S", name=f"S{j}")
            g2 = eppool.tile([128, 1024], BF16, tag="G", name=f"G{j}")
            e2 = eppool.tile([128, 1024], BF16, tag="E", name=f"E{j}")
            if last:
                # split the chain into 512-wide halves on the DVE (GpSimd
                # cannot read PSUM and is ~2x slower on tensor_tensor) so
                # the bh=0 half's Erf/out can start one op earlier.
                for bh in range(2):
                    sl = slice(bh * 512, (bh + 1) * 512)
                    nc.vector.tensor_tensor(
                        s2[:, sl], psA[:, sl], mu_js[j][:, sl], OP.add
                    )
                nc.scalar.activation(
                    e2[:], psB[:], AF.Exp, bias=nln2_sb[:], scale=-(2.0**-12)
                )
                for bh in range(2):
                    sl = slice(bh * 512, (bh + 1) * 512)
                    nc.vector.tensor_tensor(g2[:, sl], s2[:, sl], e2[:, sl], OP.mult)
            else:
                nc.vector.tensor_tensor(s2[:], psA[:], mu_js[j][:], OP.add)
                nc.scalar.activation(
                    e2[:], psB[:], AF.Exp, bias=nln2_sb[:], scale=-(2.0**-12)
                )
                nc.vector.tensor_tensor(g2[:], s2[:], e2[:], OP.mult)
            g2s[j] = g2
            pend_erf.append(j)
            if j in ERF_FLUSH_AFTER:
                flush_erfs(pend_erf[:-1])
                del pend_erf[:-1]
        flush_erfs(pend_erf)


_NC = None
_last_in_maps = None


def kernel(mu, t, gamma, W1, b1, W2, b2):
    global _NC
    if _NC is None:
        _NC = _build()
    nc = _NC

    f16 = np.float16
    f32 = np.float32

    def q8(a, scale):
        return np.clip(np.asarray(a, f32) * scale, -240.0, 240.0).astype(NPF8)

    # x^T = mu^T * 2^4 fp8, pair-packed [128, (q, r, b)] flat; the t column
    # is a rank-2 seed (with b1)
    Xt = q8(mu, SX).T                      # (D, B)
    w1_np = np.ascontiguousarray(
        q8(W1[: D], SW).reshape(KP1, 2, 128, H).transpose(2, 0, 1, 3)
    ).reshape(128, KP1 * 2 * H)
    seedw_np = np.stack([q8(b1, SX * SW), q8(W1[D], SW)])  # [2, H]

    b2_64 = np.asarray(b2, np.float64)
    b2A, b2B = b2_64[:D], b2_64[D:]
    EB = np.exp(-b2B)  # fold b2 of the B half as a per-col factor on W2A
    W2f = np.asarray(W2, f32).astype(np.float64)
    W2q = np.concatenate([W2f[:, :D] * EB[None, :], W2f[:, D:]], axis=1)
    # W2 pack [p, j, k, half, col] = W2q[k*128+p, half*D + j*512 + col]
    w2_np = np.ascontiguousarray(
        q8(W2q, SW).reshape(KC2, 128, 2, NJ, 512).transpose(1, 3, 0, 2, 4)
    )

    g64 = np.asarray(gamma, dtype=np.float64)[:, 0]
    s64 = np.sqrt((1.0 - g64) / g64)
    qm_t = -1.0 / (g64 * s64)
    qa_t = 0.875 / s64
    # M = 2^12 * (mu*qm + qa + b2A) * EB, bf16  (absmax ~4e5, well in range)
    mun2 = (
        (
            np.asarray(mu, np.float64) * qm_t[:, None]
            + qa_t[:, None]
            + b2A[None, :]
        )
        * EB[None, :]
        * (SX * SW)
    ).astype(NPBF16)
    t8 = q8(t, SX)  # (B, 1)

    in_maps = []
    for c in range(NCORES):
        sl = slice(c * BS, (c + 1) * BS)
        in_maps.append(
            {
                "xT": np.ascontiguousarray(
                    Xt[:, sl].reshape(KP1, 2, 128, BS).transpose(2, 0, 1, 3)
                ).reshape(128, KP1 * 2 * BS),
                "w1": w1_np,
                "w2": w2_np,
                "seedw": seedw_np,
                "seedx": np.stack(
                    [np.ones(BS, dtype=NPF8), t8[sl, 0].astype(NPF8)]
                ),
                # [p, j, h, c] = M[h*128+p, j*512+c], flat [128, 8192]
                "mun": np.ascontiguousarray(
                    mun2[sl].reshape(2, 128, NJ, 512).transpose(1, 2, 0, 3)
                ).reshape(128, NJ * 1024),
            }
        )

    global _last_in_maps
    _last_in_maps = in_maps

    res = run_bass_kernel_spmd(nc, in_maps, core_ids=list(range(NCORES)))
    # out dram is [p, (j, h, c)]; unpack to [b, d] = [h*128+p, j*512+c]
    return np.concatenate(
        [
            r["out"]
            .reshape(128, NJ, 2, 512)
            .transpose(2, 0, 1, 3)
            .reshape(BS, D)
            .astype(np.float32)
            for r in res.results
        ],
        axis=0,
    )


# revision 45
# speedup vs baseline: 1.1436x; 1.0449x over previous
"""Trainium2 Bass kernel for nn_DiscretisedBNF (histogram binning MLP).

Math: the reference's per-bin CDF sum telescopes exactly (kl_{k+1} == kr_k
bit-identically, and cdf(kl_0) = cdf(kr_0) = 0 since those bounds are <= -1),
so

    sum_k [cdf(kr_k) - cdf(kl_k)] = cdf(kr_{K-1}) = 0.5*(1 + erf((0.875-mu_x)*inv))

with mu_x = mu/gamma - s*mu_eps, inv = 1/(sigma_x*sqrt(2)), sigma_x =
s*exp(ln_sigma_eps), s = sqrt((1-gamma)/gamma).  Rearranged for the chip,
with every per-column constant folded on the host:

    arg = (psA + M) * e              psA = 2^12 * (h @ W2A')     (PSUM)
    M   = 2^12*(mu*qm + qa + b2A)*EB   (host-precomputed, bf16)
    e   = exp(-2^-12*psB - ln(sqrt2) - 12ln2)                    (= E*2^-12)
    W2A'= W2A * EB,  EB = exp(-b2B)   (b2 of the B half folded as a
                                       multiplicative per-col factor)
    out = 0.5*erf(arg) + 0.5

Precision: both matmuls run in fp8 e4m3 with perf_mode=DoubleRow (2 fp8
weights/cell, ~2x PE rate, half the fp16 DMA bytes).  Scales keep everything
in e4m3's normal range (max +-240): x by 2^4, W1/W2 by 2^8.  The b1 bias and
the t-row of the concat([mu,t]) input are seeded into the mm1 PSUM
accumulation by a single rank-2 matmul per m-chunk (so mm1's streamed
contraction is exactly D=4096 = 16 DoubleRow pairs, and the Lrelu needs no
per-tile bias -> two merged 1024-wide Lrelus).  Epilogue intermediates are
bf16.  Measured end-to-end rel err: 1.0e-2 vs the 2e-2 gate.

Sharding: pure data parallel - batch dim (2048) split 256 rows per core;
weights replicated.  DoubleRow wants k-chunk PAIRS interleaved on the same
128 partitions (AP [128, 2, free]); the host packs accordingly.

Scheduling notes (from HW traces):
- Every bulk DMA is one contiguous multi-KB run per partition (flat 2D dram
  tensors + flat tiles, matmul views via AP rearrange): the DGE generates
  ~80 descriptors/us per queue, so small-descriptor patterns cap a ring at
  ~100-160 GB/s while 4-8KB runs reach full HBM bandwidth.
- The HAM clock gate needs a few us of PE-busy to open (warm-up burst) and
  closes again on PE idle >~2-3us; rank-2 seeds don't count as busy, so a
  few more warm-up matmuls follow them.
- The ACT engine reloads its function table on every Exp<->Erf switch
  (1.3-1.5us), so Erfs are batched ([4,2,2]) behind the Exps whose PSUM
  releases gate the next j-group's matmuls, with dummy activations
  preloading the Lrelu/Exp tables during idle windows.
- PSUM is managed as [128,1024] two-bank tiles: mm1 uses 2 (4 m-chunks
  each), mm2 one psA + one psB per j, so the epilogue runs 1024-wide ops.
- The last j computes its B half first and splits the epilogue chain into
  512-wide halves so the tail after the final matmul is short.
"""

import numpy as np
import ml_dtypes
from contextlib import ExitStack

import concourse.bass as bass
import concourse.mybir as mybir
from concourse.tile import TileContext
from concourse.tile_rust import add_dep_helper
from concourse.bass_utils import run_bass_kernel_spmd

B, D, H = 2048, 4096, 1024
NCORES = 8
BS = B // NCORES            # 256 batch rows per core
KP1 = 16                    # mm1 streamed contract pairs: 16*256 = 4096 = D
KC2 = H // 128              # 8 contract chunks for matmul2
KP2 = KC2 // 2              # 4 DoubleRow pairs
NJ = D // 512               # 8 output column groups of 512
LEAKY_SLOPE = 0.01
LN_SQRT2 = 0.34657359027997264
LN2 = 0.6931471805599453
SX = 2.0**4                 # x fp8 scale
SW = 2.0**8                 # W1/W2 fp8 scale

F8 = mybir.dt.float8e4
F16 = mybir.dt.float16
BF16 = mybir.dt.bfloat16
F32 = mybir.dt.float32
AF = mybir.ActivationFunctionType
OP = mybir.AluOpType
DR = mybir.MatmulPerfMode.DoubleRow

NPF8 = ml_dtypes.float8_e4m3
NPBF16 = ml_dtypes.bfloat16


def split_multi_waits(nc):
    """This container's walrus accepts at most ONE sync-wait per instruction
    (setupSyncWait: 'Too many sync wait commands').  Split any instruction
    carrying N>1 waits into N-1 single-wait NoOps on the same engine placed
    immediately before it."""
    cnt = 0
    sync_info_cls = None
    for f in nc.m.functions:
        for bb in f.blocks:
            out = []
            changed = False
            for inst in bb.instructions:
                si = inst.sync_info
                waits = list(si.on_wait) if si and si.on_wait else []
                if len(waits) > 1:
                    if sync_info_cls is None:
                        sync_info_cls = type(si)
                    for w in waits[:-1]:
                        nop = mybir.InstNoOp(name=f"waitsplit_{cnt}", ins=[], outs=[])
                        cnt += 1
                        nop.engine = inst.engine
                        nop.sync_info = sync_info_cls(on_wait=[w], on_update=[])
                        out.append(nop)
                    si.on_wait = waits[-1:]
                    changed = True
                out.append(inst)
            if changed:
                bb.instructions = out
    return cnt


def _lean_drain_and_barrier(self, tick_clock, wait_clock):
    """Replacement for TileContext._drain_and_barrier: drain + ONE barrier,
    skipping the ~7us semaphore-clear butterfly.  The Bass preamble re-clears
    every kernel semaphore at the start of each execution, and no sibling
    TileContext follows this one, so the tail clear is redundant.  The
    multi-wait drain is split later by split_multi_waits."""
    import concourse.tile as tile_mod

    nc = self.nc
    drain_inst = nc.sync.drain()
    wait_clock.add_sem_waits(
        drain_inst.ins, tile_mod.ScopedClock({None: tick_clock.global_clock})
    )
    # No all_engine_barrier: the SP drain above waits on every semaphore's
    # final tick (all engines' last work and all DMA completions), so SP
    # retires last and execution end implies everything finished.
    popped = nc._tile_sem_poison_stack.pop()
    assert popped is self._sem_poison


def _build():
    nc = bass.Bass()
    orig_drain = TileContext._drain_and_barrier
    TileContext._drain_and_barrier = _lean_drain_and_barrier
    try:
        _build_body(nc)
    finally:
        TileContext._drain_and_barrier = orig_drain

    split_multi_waits(nc)
    return nc


def _build_body(nc):
    # All bulk tensors are FLAT per-partition so every DMA is one contiguous
    # multi-KB run per partition: the DGE generates descriptors at a fixed
    # ~80/us per queue, so sub-2KB descriptors cap a ring at ~160 GB/s
    # (measured: W1's 4D pattern starved mm1 at ~70-125 GB/s).
    xT = nc.dram_tensor("xT", [128, KP1 * 2 * BS], F8, kind="ExternalInput")
    w1 = nc.dram_tensor("w1", [128, KP1 * 2 * H], F8, kind="ExternalInput")
    w2 = nc.dram_tensor("w2", [128, NJ, KC2, 2, 512], F8, kind="ExternalInput")
    # rank-2 seed operands: row0 = (b1*2^12, ones), row1 = (W1[4096]*2^8, t*2^4)
    seedw = nc.dram_tensor("seedw", [2, H], F8, kind="ExternalInput")
    seedx = nc.dram_tensor("seedx", [2, BS], F8, kind="ExternalInput")
    # mun/out packed [p, j, h, col] -> flat [128, 8192]; host un/packs
    mun = nc.dram_tensor("mun", [128, NJ * 1024], BF16, kind="ExternalInput")
    outd = nc.dram_tensor("out", [128, NJ * 1024], F16, kind="ExternalOutput")

    with TileContext(nc) as tc, ExitStack() as ctx:
        const = ctx.enter_context(tc.tile_pool(name="const", bufs=1))
        xpool = ctx.enter_context(tc.tile_pool(name="xpool", bufs=1))
        w1pool = ctx.enter_context(tc.tile_pool(name="w1pool", bufs=4))
        hpool = ctx.enter_context(tc.tile_pool(name="hpool", bufs=1))
        w2pool = ctx.enter_context(tc.tile_pool(name="w2pool", bufs=6))
        eppool = ctx.enter_context(tc.tile_pool(name="eppool", bufs=4))
        outpool = ctx.enter_context(tc.tile_pool(name="outpool", bufs=3))
        pspool = ctx.enter_context(tc.tile_pool(name="pspool", bufs=4, space="PSUM"))

        # --- constants ---
        ones_row = const.tile([128, BS], F16, name="ones_row")
        nc.vector.memset(ones_row[:], 1.0)
        ones128 = const.tile([128, 128], F16, name="ones128")
        nc.vector.memset(ones128[:], 1.0)
        nln2_sb = const.tile([128, 1], F32, name="nln2_sb")
        nc.vector.memset(nln2_sb[:], -(LN_SQRT2 + 12.0 * LN2))
        scratch = const.tile([128, 1], F32, name="scratch")


        # short PE warm-up: dependency-free full-rank matmuls open the HAM
        # clock gate (needs ~3.5us of PE-busy; rank-2 seeds don't count)
        ps_warm = pspool.tile([128, 1024], F32, tag="ps", name="ps_warm")

        def warmup(n):
            for _ in range(n):
                nc.tensor.matmul(
                    ps_warm[:, :BS], ones128[:], ones_row[:], start=True, stop=True
                )

        warmup(22)

        # tiny rank-2 seed operand loads FIRST on the Scalar HWDGE ring
        # (the SWDGE ring takes ~3.5us to deliver its first byte, which
        # stalled the PE between warm-up and mm1 and jittered the HAM boost)
        seedw_sb = const.tile([2, H], F8, name="seedw_sb")
        nc.scalar.dma_start(out=seedw_sb[:], in_=seedw[:])
        seedx_sb = const.tile([2, BS], F8, name="seedx_sb")
        nc.scalar.dma_start(out=seedx_sb[:], in_=seedx[:])

        # --- x^T resident, pair-packed; Scalar HWDGE ring (the SWDGE ring
        # ramps far too slowly at kernel start), one flat tile, split so
        # mm1's first pairs don't wait for the whole 1 MB.
        XT_PARTS = [4, 6, 6]  # pairs per part; front-load small
        xt_flat = xpool.tile([128, KP1 * 2 * BS], F8, name="xt_flat")
        q0 = 0
        for npair in XT_PARTS:
            nc.scalar.dma_start(
                out=xt_flat[:, q0 * 512 : (q0 + npair) * 512],
                in_=xT[:, q0 * 512 : (q0 + npair) * 512],
            )
            q0 += npair
        assert q0 == KP1

        # preload the Lrelu ACT table (1.5us load, hidden under mm1)
        nc.scalar.activation(scratch[:], nln2_sb[:], AF.Lrelu, alpha=LEAKY_SLOPE)

        def xt_pair(q):
            return xt_flat[:, q * 512 : (q + 1) * 512].rearrange(
                "p (r b) -> p r b", r=2
            )

        # --- matmul1: h^T = W1^T @ x^T, H on partitions, fp8 DoubleRow.
        # PSUM: two [128,1024] double-bank tiles, 4 m-chunks (256 cols) each.
        ps1_t = [
            pspool.tile([128, 1024], F32, tag="ps", name=f"ps1_t{i}")
            for i in range(2)
        ]

        def ps1(m):
            return ps1_t[m // 4][:, (m % 4) * BS : (m % 4 + 1) * BS]

        # rank-2 seeds (b1 bias + t-row contribution) start the mm1
        # accumulation; a few more warm-up matmuls after them bridge the
        # HAM-invisible window (rank-2 work doesn't count as PE-busy).
        for m in range(KC2):
            nc.tensor.matmul(
                ps1(m),
                seedw_sb[:, m * 128 : (m + 1) * 128],
                seedx_sb[:],
                start=True,
                stop=False,
            )
        warmup(4)

        W1_PARTS = [1, 1, 2, 4, 4, 4]  # pairs per group, Sync ring, flat tiles
        PW = 2 * H  # elements per pair per partition
        mm1_last = {}
        q = 0
        for g, npair in enumerate(W1_PARTS):
            w1g = w1pool.tile(
                [128, max(W1_PARTS) * PW], F8, tag="w1t", name=f"w1g{g}"
            )
            nc.sync.dma_start(
                out=w1g[:, : npair * PW],
                in_=w1[:, q * PW : (q + npair) * PW],
            )
            for i in range(npair):
                rhs = xt_pair(q)
                w1p = w1g[:, i * PW : (i + 1) * PW].rearrange(
                    "p (r h) -> p r h", r=2
                )
                for m in range(KC2):
                    mm = nc.tensor.matmul(
                        ps1(m),
                        w1p[:, :, m * 128 : (m + 1) * 128],
                        rhs,
                        start=False,
                        stop=(q == KP1 - 1),
                        perf_mode=DR,
                    )
                mm1_last[q] = mm
                q += 1
        assert q == KP1

        # h fp8 at 2^4 scale, all 8 chunks in one [128, 8, 256] tile;
        # Lrelu(2^-8 * psum) = 2^4 * Lrelu(xW1 + b1): bias was seeded, so
        # TWO merged 1024-wide Lrelus cover all 8 chunks.
        h_all = hpool.tile([128, KC2, BS], F8, name="h_all")
        for i in range(2):
            nc.scalar.activation(
                h_all[:, 4 * i : 4 * (i + 1), :],
                ps1_t[i][:],
                AF.Lrelu,
                scale=2.0**-8,
                alpha=LEAKY_SLOPE,
            )
        # preload the Exp table while mm2's first matmuls run
        nc.scalar.activation(scratch[:], nln2_sb[:], AF.Exp)

        # --- matmul2 + fused epilogue ---
        w2ts, mu_js = {}, {}
        for j in range(NJ):
            w2t = w2pool.tile([128, KC2, 2, 512], F8, tag="w2", name=f"w2t{j}")
            pace = {0: 7, 1: 10, 2: 13, 3: 15}.get(j)
            dma = nc.gpsimd.dma_start(out=w2t[:], in_=w2[:, j])
            if pace is not None:
                add_dep_helper(dma.ins, mm1_last[pace].ins, True, "pace w2")
            w2ts[j] = w2t
            mu_j = eppool.tile([128, 1024], BF16, tag="mu", name=f"mu{j}")
            dma = nc.scalar.dma_start(
                out=mu_j[:], in_=mun[:, j * 1024 : (j + 1) * 1024]
            )
            if j == 0:
                add_dep_helper(dma.ins, mm1_last[13].ins, True, "pace mu")
            mu_js[j] = mu_j

        def emit_matmuls(j, b_first):
            w2t = w2ts[j]
            psA = pspool.tile([128, 1024], F32, tag="ps", name=f"psA{j}")
            psB = pspool.tile([128, 1024], F32, tag="ps", name=f"psB{j}")
            dst = {0: psA, 1: psB}
            for h in ((1, 0) if b_first else (0, 1)):
                for qq in range(KP2):
                    for bh in range(2):
                        nc.tensor.matmul(
                            dst[h][:, bh * 512 : (bh + 1) * 512],
                            h_all[:, 2 * qq : 2 * qq + 2, bh * 128 : (bh + 1) * 128],
                            w2t[:, 2 * qq : 2 * qq + 2, h, :],
                            start=(qq == 0),
                            stop=(qq == KP2 - 1),
                            perf_mode=DR,
                        )
            return psA, psB

        g2s = {}
        pend_erf = []

        def flush_erfs(js):
            for j in js:
                o2 = outpool.tile([128, 1024], F16, tag="o", name=f"O{j}")
                if j == NJ - 1:
                    # split the last chain across ACT/DVE/GpSimd halves so
                    # the tail after the final matmul is ~half as deep
                    for bh in range(2):
                        sl = slice(bh * 512, (bh + 1) * 512)
                        r2 = eppool.tile([128, 512], BF16, tag="R", name=f"R{j}_{bh}")
                        nc.scalar.activation(r2[:], g2s[j][:, sl], AF.Erf)
                        eng = nc.gpsimd if bh == 0 else nc.vector
                        eng.tensor_scalar(o2[:, sl], r2[:], 0.5, 0.5, OP.mult, OP.add)
                        nc.sync.dma_start(
                            out=outd[:, j * 1024 + bh * 512 : j * 1024 + (bh + 1) * 512],
                            in_=o2[:, sl],
                        )
                else:
                    r2 = eppool.tile([128, 1024], BF16, tag="Rw", name=f"R{j}")
                    nc.scalar.activation(r2[:], g2s[j][:], AF.Erf)
                    nc.gpsimd.tensor_scalar(o2[:], r2[:], 0.5, 0.5, OP.mult, OP.add)
                    nc.sync.dma_start(
                        out=outd[:, j * 1024 : (j + 1) * 1024], in_=o2[:]
                    )

        # Flush AFTER j4/j6 so each Erf batch sits behind the Exp whose PSUM
        # release the j+2 matmuls need — an Erf batch queued before that Exp
        # stalls the PE on psB slots.
        ERF_FLUSH_AFTER = {4, 6}  # Erf batches [4,2,2]

        for j in range(NJ):
            last = j == NJ - 1
            psA, psB = emit_matmuls(j, b_first=last)
            s2 = eppool.tile([128, 1024], BF16, tag="S", name=f"S{j}")
            g2 = eppool.tile([128, 1024], BF16, tag="G", name=f"G{j}")
            e2 = eppool.tile([128, 1024], BF16, tag="E", name=f"E{j}")
            if last:
                # split the chain into 512-wide halves on the DVE (GpSimd
                # cannot read PSUM and is ~2x slower on tensor_tensor) so
                # the bh=0 half's Erf/out can start one op earlier.
                for bh in range(2):
                    sl = slice(bh * 512, (bh + 1) * 512)
                    nc.vector.tensor_tensor(
                        s2[:, sl], psA[:, sl], mu_js[j][:, sl], OP.add
                    )
                nc.scalar.activation(
                    e2[:], psB[:], AF.Exp, bias=nln2_sb[:], scale=-(2.0**-12)
                )
                for bh in range(2):
                    sl = slice(bh * 512, (bh + 1) * 512)
                    nc.vector.tensor_tensor(g2[:, sl], s2[:, sl], e2[:, sl], OP.mult)
            else:
                nc.vector.tensor_tensor(s2[:], psA[:], mu_js[j][:], OP.add)
                nc.scalar.activation(
                    e2[:], psB[:], AF.Exp, bias=nln2_sb[:], scale=-(2.0**-12)
                )
                nc.vector.tensor_tensor(g2[:], s2[:], e2[:], OP.mult)
            g2s[j] = g2
            pend_erf.append(j)
            if j in ERF_FLUSH_AFTER:
                flush_erfs(pend_erf[:-1])
                del pend_erf[:-1]
        flush_erfs(pend_erf)


_NC = None
_last_in_maps = None


def kernel(mu, t, gamma, W1, b1, W2, b2):
    global _NC
    if _NC is None:
        _NC = _build()
    nc = _NC

    f16 = np.float16
    f32 = np.float32

    def q8(a, scale):
        return np.clip(np.asarray(a, f32) * scale, -240.0, 240.0).astype(NPF8)

    # x^T = mu^T * 2^4 fp8, pair-packed [128, (q, r, b)] flat; the t column
    # is a rank-2 seed (with b1)
    Xt = q8(mu, SX).T                      # (D, B)
    w1_np = np.ascontiguousarray(
        q8(W1[: D], SW).reshape(KP1, 2, 128, H).transpose(2, 0, 1, 3)
    ).reshape(128, KP1 * 2 * H)
    seedw_np = np.stack([q8(b1, SX * SW), q8(W1[D], SW)])  # [2, H]

    b2_64 = np.asarray(b2, np.float64)
    b2A, b2B = b2_64[:D], b2_64[D:]
    EB = np.exp(-b2B)  # fold b2 of the B half as a per-col factor on W2A
    W2f = np.asarray(W2, f32).astype(np.float64)
    W2q = np.concatenate([W2f[:, :D] * EB[None, :], W2f[:, D:]], axis=1)
    # W2 pack [p, j, k, half, col] = W2q[k*128+p, half*D + j*512 + col]
    w2_np = np.ascontiguousarray(
        q8(W2q, SW).reshape(KC2, 128, 2, NJ, 512).transpose(1, 3, 0, 2, 4)
    )

    g64 = np.asarray(gamma, dtype=np.float64)[:, 0]
    s64 = np.sqrt((1.0 - g64) / g64)
    qm_t = -1.0 / (g64 * s64)
    qa_t = 0.875 / s64
    # M = 2^12 * (mu*qm + qa + b2A) * EB, bf16  (absmax ~4e5, well in range)
    mun2 = (
        (
            np.asarray(mu, np.float64) * qm_t[:, None]
            + qa_t[:, None]
            + b2A[None, :]
        )
        * EB[None, :]
        * (SX * SW)
    ).astype(NPBF16)
    t8 = q8(t, SX)  # (B, 1)

    in_maps = []
    for c in range(NCORES):
        sl = slice(c * BS, (c + 1) * BS)
        in_maps.append(
            {
                "xT": np.ascontiguousarray(
                    Xt[:, sl].reshape(KP1, 2, 128, BS).transpose(2, 0, 1, 3)
                ).reshape(128, KP1 * 2 * BS),
                "w1": w1_np,
                "w2": w2_np,
                "seedw": seedw_np,
                "seedx": np.stack(
                    [np.ones(BS, dtype=NPF8), t8[sl, 0].astype(NPF8)]
                ),
                # [p, j, h, c] = M[h*128+p, j*512+c], flat [128, 8192]
                "mun": np.ascontiguousarray(
                    mun2[sl].reshape(2, 128, NJ, 512).transpose(1, 2, 0, 3)
                ).reshape(128, NJ * 1024),
            }
        )

    global _last_in_maps
    _last_in_maps = in_maps

    res = run_bass_kernel_spmd(nc, in_maps, core_ids=list(range(NCORES)))
    # out dram is [p, (j, h, c)]; unpack to [b, d] = [h*128+p, j*512+c]
    return np.concatenate(
        [
            r["out"]
            .reshape(128, NJ, 2, 512)
            .transpose(2, 0, 1, 3)
            .reshape(BS, D)
            .astype(np.float32)
            for r in res.results
        ],
        axis=0,
    )


# revision 47
# speedup vs baseline: 1.1636x; 1.0175x over previous
"""Trainium2 Bass kernel for nn_DiscretisedBNF (histogram binning MLP).

Math: the reference's per-bin CDF sum telescopes exactly (kl_{k+1} == kr_k
bit-identically, and cdf(kl_0) = cdf(kr_0) = 0 since those bounds are <= -1),
so

    sum_k [cdf(kr_k) - cdf(kl_k)] = cdf(kr_{K-1}) = 0.5*(1 + erf((0.875-mu_x)*inv))

with mu_x = mu/gamma - s*mu_eps, inv = 1/(sigma_x*sqrt(2)), sigma_x =
s*exp(ln_sigma_eps), s = sqrt((1-gamma)/gamma).  Rearranged for the chip,
with every per-column constant folded on the host:

    arg = (psA + M) * e              psA = 2^12 * (h @ W2A')     (PSUM)
    M   = 2^12*(mu*qm + qa + b2A)*EB   (host-precomputed, bf16)
    e   = exp(-2^-12*psB - ln(sqrt2) - 12ln2)                    (= E*2^-12)
    W2A'= W2A * EB,  EB = exp(-b2B)   (b2 of the B half folded as a
                                       multiplicative per-col factor)
    out = 0.5*erf(arg) + 0.5

Precision: both matmuls run in fp8 e4m3 with perf_mode=DoubleRow (2 fp8
weights/cell, ~2x PE rate, half the fp16 DMA bytes).  Scales keep everything
in e4m3's normal range (max +-240): x by 2^4, W1/W2 by 2^8.  The b1 bias and
the t-row of the concat([mu,t]) input are seeded into the mm1 PSUM
accumulation by a single rank-2 matmul per m-chunk (so mm1's streamed
contraction is exactly D=4096 = 16 DoubleRow pairs, and the Lrelu needs no
per-tile bias -> two merged 1024-wide Lrelus).  Epilogue intermediates are
bf16.  Measured end-to-end rel err: 1.0e-2 vs the 2e-2 gate.

Sharding: pure data parallel - batch dim (2048) split 256 rows per core;
weights replicated.  DoubleRow wants k-chunk PAIRS interleaved on the same
128 partitions (AP [128, 2, free]); the host packs accordingly.

Scheduling notes (from HW traces):
- Every bulk DMA is one contiguous multi-KB run per partition (flat 2D dram
  tensors + flat tiles, matmul views via AP rearrange): the DGE generates
  ~80 descriptors/us per queue, so small-descriptor patterns cap a ring at
  ~100-160 GB/s while 4-8KB runs reach full HBM bandwidth.
- The HAM clock gate needs a few us of PE-busy to open (warm-up burst) and
  closes again on PE idle >~2-3us; rank-2 seeds don't count as busy, so a
  few more warm-up matmuls follow them.
- The ACT engine reloads its function table on every Exp<->Erf switch
  (1.3-1.5us), so Erfs are batched ([4,2,2]) behind the Exps whose PSUM
  releases gate the next j-group's matmuls, with dummy activations
  preloading the Lrelu/Exp tables during idle windows.
- PSUM is managed as [128,1024] two-bank tiles: mm1 uses 2 (4 m-chunks
  each), mm2 one psA + one psB per j, so the epilogue runs 1024-wide ops.
- The last j computes its B half first and splits the epilogue chain into
  512-wide halves so the tail after the final matmul is short.
"""

import numpy as np
import ml_dtypes
from contextlib import ExitStack

import concourse.bass as bass
import concourse.mybir as mybir
from concourse.tile import TileContext
from concourse.tile_rust import add_dep_helper
from concourse.bass_utils import run_bass_kernel_spmd

B, D, H = 2048, 4096, 1024
NCORES = 8
BS = B // NCORES            # 256 batch rows per core
KP1 = 16                    # mm1 streamed contract pairs: 16*256 = 4096 = D
KC2 = H // 128              # 8 contract chunks for matmul2
KP2 = KC2 // 2              # 4 DoubleRow pairs
NJ = D // 512               # 8 output column groups of 512
LEAKY_SLOPE = 0.01
LN_SQRT2 = 0.34657359027997264
LN2 = 0.6931471805599453
SX = 2.0**4                 # x fp8 scale
SW = 2.0**8                 # W1/W2 fp8 scale

F8 = mybir.dt.float8e4
F16 = mybir.dt.float16
BF16 = mybir.dt.bfloat16
F32 = mybir.dt.float32
AF = mybir.ActivationFunctionType
OP = mybir.AluOpType
DR = mybir.MatmulPerfMode.DoubleRow

NPF8 = ml_dtypes.float8_e4m3
NPBF16 = ml_dtypes.bfloat16


def split_multi_waits(nc):
    """This container's walrus accepts at most ONE sync-wait per instruction
    (setupSyncWait: 'Too many sync wait commands').  Split any instruction
    carrying N>1 waits into N-1 single-wait NoOps on the same engine placed
    immediately before it."""
    cnt = 0
    sync_info_cls = None
    for f in nc.m.functions:
        for bb in f.blocks:
            out = []
            changed = False
            for inst in bb.instructions:
                si = inst.sync_info
                waits = list(si.on_wait) if si and si.on_wait else []
                if len(waits) > 1:
                    if sync_info_cls is None:
                        sync_info_cls = type(si)
                    for w in waits[:-1]:
                        nop = mybir.InstNoOp(name=f"waitsplit_{cnt}", ins=[], outs=[])
                        cnt += 1
                        nop.engine = inst.engine
                        nop.sync_info = sync_info_cls(on_wait=[w], on_update=[])
                        out.append(nop)
                    si.on_wait = waits[-1:]
                    changed = True
                out.append(inst)
            if changed:
                bb.instructions = out
    return cnt


def _lean_drain_and_barrier(self, tick_clock, wait_clock):
    """Replacement for TileContext._drain_and_barrier: drain + ONE barrier,
    skipping the ~7us semaphore-clear butterfly.  The Bass preamble re-clears
    every kernel semaphore at the start of each execution, and no sibling
    TileContext follows this one, so the tail clear is redundant.  The
    multi-wait drain is split later by split_multi_waits."""
    import concourse.tile as tile_mod

    nc = self.nc
    drain_inst = nc.sync.drain()
    wait_clock.add_sem_waits(
        drain_inst.ins, tile_mod.ScopedClock({None: tick_clock.global_clock})
    )
    # No all_engine_barrier: the SP drain above waits on every semaphore's
    # final tick (all engines' last work and all DMA completions), so SP
    # retires last and execution end implies everything finished.
    popped = nc._tile_sem_poison_stack.pop()
    assert popped is self._sem_poison


def _build():
    nc = bass.Bass()
    orig_drain = TileContext._drain_and_barrier
    TileContext._drain_and_barrier = _lean_drain_and_barrier
    try:
        _build_body(nc)
    finally:
        TileContext._drain_and_barrier = orig_drain

    split_multi_waits(nc)
    return nc


def _build_body(nc):
    # All bulk tensors are FLAT per-partition so every DMA is one contiguous
    # multi-KB run per partition: the DGE generates descriptors at a fixed
    # ~80/us per queue, so sub-2KB descriptors cap a ring at ~160 GB/s
    # (measured: W1's 4D pattern starved mm1 at ~70-125 GB/s).
    xT = nc.dram_tensor("xT", [128, KP1 * 2 * BS], F8, kind="ExternalInput")
    w1 = nc.dram_tensor("w1", [128, KP1 * 2 * H], F8, kind="ExternalInput")
    w2 = nc.dram_tensor("w2", [128, NJ, KC2, 2, 512], F8, kind="ExternalInput")
    # rank-2 seed operands: row0 = (b1*2^12, ones), row1 = (W1[4096]*2^8, t*2^4)
    seedw = nc.dram_tensor("seedw", [2, H], F8, kind="ExternalInput")
    seedx = nc.dram_tensor("seedx", [2, BS], F8, kind="ExternalInput")
    # mun/out packed [p, j, h, col] -> flat [128, 8192]; host un/packs
    mun = nc.dram_tensor("mun", [128, NJ * 1024], BF16, kind="ExternalInput")
    outd = nc.dram_tensor("out", [128, NJ * 1024], F16, kind="ExternalOutput")

    with TileContext(nc) as tc, ExitStack() as ctx:
        const = ctx.enter_context(tc.tile_pool(name="const", bufs=1))
        xpool = ctx.enter_context(tc.tile_pool(name="xpool", bufs=1))
        w1pool = ctx.enter_context(tc.tile_pool(name="w1pool", bufs=4))
        hpool = ctx.enter_context(tc.tile_pool(name="hpool", bufs=1))
        w2pool = ctx.enter_context(tc.tile_pool(name="w2pool", bufs=6))
        eppool = ctx.enter_context(tc.tile_pool(name="eppool", bufs=4))
        outpool = ctx.enter_context(tc.tile_pool(name="outpool", bufs=3))
        pspool = ctx.enter_context(tc.tile_pool(name="pspool", bufs=4, space="PSUM"))

        # --- constants ---
        ones_row = const.tile([128, BS], F16, name="ones_row")
        nc.vector.memset(ones_row[:], 1.0)
        ones128 = const.tile([128, 128], F16, name="ones128")
        nc.vector.memset(ones128[:], 1.0)
        nln2_sb = const.tile([128, 1], F32, name="nln2_sb")
        nc.vector.memset(nln2_sb[:], -(LN_SQRT2 + 12.0 * LN2))
        scratch = const.tile([128, 1], F32, name="scratch")


        # short PE warm-up: dependency-free full-rank matmuls open the HAM
        # clock gate (needs ~3.5us of PE-busy; rank-2 seeds don't count)
        ps_warm = pspool.tile([128, 1024], F32, tag="ps", name="ps_warm")

        def warmup(n):
            for _ in range(n):
                nc.tensor.matmul(
                    ps_warm[:, :BS], ones128[:], ones_row[:], start=True, stop=True
                )

        warmup(22)

        # tiny rank-2 seed operand loads FIRST on the Scalar HWDGE ring
        # (the SWDGE ring takes ~3.5us to deliver its first byte, which
        # stalled the PE between warm-up and mm1 and jittered the HAM boost)
        seedw_sb = const.tile([2, H], F8, name="seedw_sb")
        nc.scalar.dma_start(out=seedw_sb[:], in_=seedw[:])
        seedx_sb = const.tile([2, BS], F8, name="seedx_sb")
        nc.scalar.dma_start(out=seedx_sb[:], in_=seedx[:])

        # --- x^T resident, pair-packed; Scalar HWDGE ring (the SWDGE ring
        # ramps far too slowly at kernel start), one flat tile, split so
        # mm1's first pairs don't wait for the whole 1 MB.
        XT_PARTS = [4, 6, 6]  # pairs per part; front-load small
        xt_flat = xpool.tile([128, KP1 * 2 * BS], F8, name="xt_flat")
        q0 = 0
        for npair in XT_PARTS:
            nc.scalar.dma_start(
                out=xt_flat[:, q0 * 512 : (q0 + npair) * 512],
                in_=xT[:, q0 * 512 : (q0 + npair) * 512],
            )
            q0 += npair
        assert q0 == KP1

        # preload the Lrelu ACT table (1.5us load, hidden under mm1)
        nc.scalar.activation(scratch[:], nln2_sb[:], AF.Lrelu, alpha=LEAKY_SLOPE)

        def xt_pair(q):
            return xt_flat[:, q * 512 : (q + 1) * 512].rearrange(
                "p (r b) -> p r b", r=2
            )

        # --- matmul1: h^T = W1^T @ x^T, H on partitions, fp8 DoubleRow.
        # PSUM: two [128,1024] double-bank tiles, 4 m-chunks (256 cols) each.
        ps1_t = [
            pspool.tile([128, 1024], F32, tag="ps", name=f"ps1_t{i}")
            for i in range(2)
        ]

        def ps1(m):
            return ps1_t[m // 4][:, (m % 4) * BS : (m % 4 + 1) * BS]

        # rank-2 seeds (b1 bias + t-row contribution) start the mm1
        # accumulation; a few more warm-up matmuls after them bridge the
        # HAM-invisible window (rank-2 work doesn't count as PE-busy).
        for m in range(KC2):
            nc.tensor.matmul(
                ps1(m),
                seedw_sb[:, m * 128 : (m + 1) * 128],
                seedx_sb[:],
                start=True,
                stop=False,
            )
        warmup(4)

        W1_PARTS = [1, 1, 2, 4, 4, 4]  # pairs per group, Sync ring, flat tiles
        PW = 2 * H  # elements per pair per partition
        mm1_last = {}
        q = 0
        for g, npair in enumerate(W1_PARTS):
            w1g = w1pool.tile(
                [128, max(W1_PARTS) * PW], F8, tag="w1t", name=f"w1g{g}"
            )
            nc.sync.dma_start(
                out=w1g[:, : npair * PW],
                in_=w1[:, q * PW : (q + npair) * PW],
            )
            for i in range(npair):
                rhs = xt_pair(q)
                w1p = w1g[:, i * PW : (i + 1) * PW].rearrange(
                    "p (r h) -> p r h", r=2
                )
                for m in range(KC2):
                    mm = nc.tensor.matmul(
                        ps1(m),
                        w1p[:, :, m * 128 : (m + 1) * 128],
                        rhs,
                        start=False,
                        stop=(q == KP1 - 1),
                        perf_mode=DR,
                    )
                mm1_last[q] = mm
                q += 1
        assert q == KP1

        # h fp8 at 2^4 scale, all 8 chunks in one [128, 8, 256] tile;
        # Lrelu(2^-8 * psum) = 2^4 * Lrelu(xW1 + b1): bias was seeded, so
        # TWO merged 1024-wide Lrelus cover all 8 chunks.
        h_all = hpool.tile([128, KC2, BS], F8, name="h_all")
        for i in range(2):
            nc.scalar.activation(
                h_all[:, 4 * i : 4 * (i + 1), :],
                ps1_t[i][:],
                AF.Lrelu,
                scale=2.0**-8,
                alpha=LEAKY_SLOPE,
            )
        # preload the Exp table while mm2's first matmuls run
        nc.scalar.activation(scratch[:], nln2_sb[:], AF.Exp)

        # --- matmul2 + fused epilogue ---
        w2ts, mu_js = {}, {}
        for j in range(NJ):
            w2t = w2pool.tile([128, KC2, 2, 512], F8, tag="w2", name=f"w2t{j}")
            pace = {0: 7, 1: 10, 2: 13, 3: 15}.get(j)
            dma = nc.gpsimd.dma_start(out=w2t[:], in_=w2[:, j])
            if pace is not None:
                add_dep_helper(dma.ins, mm1_last[pace].ins, True, "pace w2")
            w2ts[j] = w2t
            mu_j = eppool.tile([128, 1024], BF16, tag="mu", name=f"mu{j}")
            dma = nc.scalar.dma_start(
                out=mu_j[:], in_=mun[:, j * 1024 : (j + 1) * 1024]
            )
            if j == 0:
                add_dep_helper(dma.ins, mm1_last[13].ins, True, "pace mu")
            mu_js[j] = mu_j

        def emit_matmuls(j, b_first):
            w2t = w2ts[j]
            psA = pspool.tile([128, 1024], F32, tag="ps", name=f"psA{j}")
            psB = pspool.tile([128, 1024], F32, tag="ps", name=f"psB{j}")
            dst = {0: psA, 1: psB}
            for h in ((1, 0) if b_first else (0, 1)):
                for qq in range(KP2):
                    for bh in range(2):
                        nc.tensor.matmul(
                            dst[h][:, bh * 512 : (bh + 1) * 512],
                            h_all[:, 2 * qq : 2 * qq + 2, bh * 128 : (bh + 1) * 128],
                            w2t[:, 2 * qq : 2 * qq + 2, h, :],
                            start=(qq == 0),
                            stop=(qq == KP2 - 1),
                            perf_mode=DR,
                        )
            return psA, psB

        g2s = {}
        pend_erf = []

        def flush_erfs(js):
            for j in js:
                o2 = outpool.tile([128, 1024], F16, tag="o", name=f"O{j}")
                if j == NJ - 1:
                    # split the last chain across ACT/DVE/GpSimd halves so
                    # the tail after the final matmul is ~half as deep
                    for bh in range(2):
                        sl = slice(bh * 512, (bh + 1) * 512)
                        r2 = eppool.tile([128, 512], BF16, tag="R", name=f"R{j}_{bh}")
                        nc.scalar.activation(r2[:], g2s[j][:, sl], AF.Erf)
                        eng = nc.gpsimd if bh == 0 else nc.vector
                        eng.tensor_scalar(o2[:, sl], r2[:], 0.5, 0.5, OP.mult, OP.add)
                        nc.sync.dma_start(
                            out=outd[:, j * 1024 + bh * 512 : j * 1024 + (bh + 1) * 512],
                            in_=o2[:, sl],
                        )
                else:
                    r2 = eppool.tile([128, 1024], BF16, tag="Rw", name=f"R{j}")
                    nc.scalar.activation(r2[:], g2s[j][:], AF.Erf)
                    nc.gpsimd.tensor_scalar(o2[:], r2[:], 0.5, 0.5, OP.mult, OP.add)
                    nc.sync.dma_start(
                        out=outd[:, j * 1024 : (j + 1) * 1024], in_=o2[:]
                    )

        # Flush AFTER j4/j6 so each Erf batch sits behind the Exp whose PSUM
        # release the j+2 matmuls need — an Erf batch queued before that Exp
        # stalls the PE on psB slots.
        ERF_FLUSH_AFTER = {4, 6}  # Erf batches [4,2,2]

        for j in range(NJ):
            last = j == NJ - 1
            psA, psB = emit_matmuls(j, b_first=last)
            s2 = eppool.tile([128, 1024], BF16, tag="S", name=f"S{j}")
            g2 = eppool.tile([128, 1024], BF16, tag="G", name=f"G{j}")
            e2 = eppool.tile([128, 1024], BF16, tag="E", name=f"E{j}")
            if last:
                # split the chain into 512-wide halves on the DVE (GpSimd
                # cannot read PSUM and is ~2x slower on tensor_tensor) so
                # the bh=0 half's Erf/out can start one op earlier.
                for bh in range(2):
                    sl = slice(bh * 512, (bh + 1) * 512)
                    nc.vector.tensor_tensor(
                        s2[:, sl], psA[:, sl], mu_js[j][:, sl], OP.add
                    )
                nc.scalar.activation(
                    e2[:], psB[:], AF.Exp, bias=nln2_sb[:], scale=-(2.0**-12)
                )
                for bh in range(2):
                    sl = slice(bh * 512, (bh + 1) * 512)
                    nc.vector.tensor_tensor(g2[:, sl], s2[:, sl], e2[:, sl], OP.mult)
            else:
                nc.vector.tensor_tensor(s2[:], psA[:], mu_js[j][:], OP.add)
                nc.scalar.activation(
                    e2[:], psB[:], AF.Exp, bias=nln2_sb[:], scale=-(2.0**-12)
                )
                nc.vector.tensor_tensor(g2[:], s2[:], e2[:], OP.mult)
            g2s[j] = g2
            pend_erf.append(j)
            if j in ERF_FLUSH_AFTER:
                flush_erfs(pend_erf[:-1])
                del pend_erf[:-1]
        flush_erfs(pend_erf)


_NC = None
_last_in_maps = None


def kernel(mu, t, gamma, W1, b1, W2, b2):
    global _NC
    if _NC is None:
        _NC = _build()
    nc = _NC

    f16 = np.float16
    f32 = np.float32

    def q8(a, scale):
        return np.clip(np.asarray(a, f32) * scale, -240.0, 240.0).astype(NPF8)

    # x^T = mu^T * 2^4 fp8, pair-packed [128, (q, r, b)] flat; the t column
    # is a rank-2 seed (with b1)
    Xt = q8(mu, SX).T                      # (D, B)
    w1_np = np.ascontiguousarray(
        q8(W1[: D], SW).reshape(KP1, 2, 128, H).transpose(2, 0, 1, 3)
    ).reshape(128, KP1 * 2 * H)
    seedw_np = np.stack([q8(b1, SX * SW), q8(W1[D], SW)])  # [2, H]

    b2_64 = np.asarray(b2, np.float64)
    b2A, b2B = b2_64[:D], b2_64[D:]
    EB = np.exp(-b2B)  # fold b2 of the B half as a per-col factor on W2A
    W2f = np.asarray(W2, f32).astype(np.float64)
    W2q = np.concatenate([W2f[:, :D] * EB[None, :], W2f[:, D:]], axis=1)
    # W2 pack [p, j, k, half, col] = W2q[k*128+p, half*D + j*512 + col]
    w2_np = np.ascontiguousarray(
        q8(W2q, SW).reshape(KC2, 128, 2, NJ, 512).transpose(1, 3, 0, 2, 4)
    )

    g64 = np.asarray(gamma, dtype=np.float64)[:, 0]
    s64 = np.sqrt((1.0 - g64) / g64)
    qm_t = -1.0 / (g64 * s64)
    qa_t = 0.875 / s64
    # M = 2^12 * (mu*qm + qa + b2A) * EB, bf16  (absmax ~4e5, well in range)
    mun2 = (
        (
            np.asarray(mu, np.float64) * qm_t[:, None]
            + qa_t[:, None]
            + b2A[None, :]
        )
        * EB[None, :]
        * (SX * SW)
    ).astype(NPBF16)
    t8 = q8(t, SX)  # (B, 1)

    in_maps = []
    for c in range(NCORES):
        sl = slice(c * BS, (c + 1) * BS)
        in_maps.append(
            {
                "xT": np.ascontiguousarray(
                    Xt[:, sl].reshape(KP1, 2, 128, BS).transpose(2, 0, 1, 3)
                ).reshape(128, KP1 * 2 * BS),
                "w1": w1_np,
                "w2": w2_np,
                "seedw": seedw_np,
                "seedx": np.stack(
                    [np.ones(BS, dtype=NPF8), t8[sl, 0].astype(NPF8)]
                ),
                # [p, j, h, c] = M[h*128+p, j*512+c], flat [128, 8192]
                "mun": np.ascontiguousarray(
                    mun2[sl].reshape(2, 128, NJ, 512).transpose(1, 2, 0, 3)
                ).reshape(128, NJ * 1024),
            }
        )

    global _last_in_maps
    _last_in_maps = in_maps

    res = run_bass_kernel_spmd(nc, in_maps, core_ids=list(range(NCORES)))
    # out dram is [p, (j, h, c)]; unpack to [b, d] = [h*128+p, j*512+c]
    return np.concatenate(
        [
            r["out"]
            .reshape(128, NJ, 2, 512)
            .transpose(2, 0, 1, 3)
            .reshape(BS, D)
            .astype(np.float32)
            for r in res.results
        ],
        axis=0,
    )


# revision 49
# speedup vs baseline: 1.1789x; 1.0132x over previous
"""Trainium2 Bass kernel for nn_DiscretisedBNF (histogram binning MLP).

Math: the reference's per-bin CDF sum telescopes exactly (kl_{k+1} == kr_k
bit-identically, and cdf(kl_0) = cdf(kr_0) = 0 since those bounds are <= -1),
so

    sum_k [cdf(kr_k) - cdf(kl_k)] = cdf(kr_{K-1}) = 0.5*(1 + erf((0.875-mu_x)*inv))

with mu_x = mu/gamma - s*mu_eps, inv = 1/(sigma_x*sqrt(2)), sigma_x =
s*exp(ln_sigma_eps), s = sqrt((1-gamma)/gamma).  Rearranged for the chip,
with every per-column constant folded on the host:

    arg = (psA + M) * e              psA = 2^12 * (h @ W2A')     (PSUM)
    M   = 2^12*(mu*qm + qa + b2A)*EB   (host-precomputed, bf16)
    e   = exp(-2^-12*psB - ln(sqrt2) - 12ln2)                    (= E*2^-12)
    W2A'= W2A * EB,  EB = exp(-b2B)   (b2 of the B half folded as a
                                       multiplicative per-col factor)
    out = 0.5*erf(arg) + 0.5

Precision: both matmuls run in fp8 e4m3 with perf_mode=DoubleRow (2 fp8
weights/cell, ~2x PE rate, half the fp16 DMA bytes).  Scales keep everything
in e4m3's normal range (max +-240): x by 2^4, W1/W2 by 2^8.  The b1 bias and
the t-row of the concat([mu,t]) input are seeded into the mm1 PSUM
accumulation by a single rank-2 matmul per m-chunk (so mm1's streamed
contraction is exactly D=4096 = 16 DoubleRow pairs, and the Lrelu needs no
per-tile bias -> two merged 1024-wide Lrelus).  Epilogue intermediates are
bf16.  Measured end-to-end rel err: 1.0e-2 vs the 2e-2 gate.

Sharding: pure data parallel - batch dim (2048) split 256 rows per core;
weights replicated.  DoubleRow wants k-chunk PAIRS interleaved on the same
128 partitions (AP [128, 2, free]); the host packs accordingly.

Scheduling notes (from HW traces):
- Every bulk DMA is one contiguous multi-KB run per partition (flat 2D dram
  tensors + flat tiles, matmul views via AP rearrange): the DGE generates
  ~80 descriptors/us per queue, so small-descriptor patterns cap a ring at
  ~100-160 GB/s while 4-8KB runs reach full HBM bandwidth.
- The HAM clock gate needs a few us of PE-busy to open (warm-up burst) and
  closes again on PE idle >~2-3us; rank-2 seeds don't count as busy, so a
  few more warm-up matmuls follow them.
- The ACT engine reloads its function table on every Exp<->Erf switch
  (1.3-1.5us), so Erfs are batched ([4,2,2]) behind the Exps whose PSUM
  releases gate the next j-group's matmuls, with dummy activations
  preloading the Lrelu/Exp tables during idle windows.
- PSUM is managed as [128,1024] two-bank tiles: mm1 uses 2 (4 m-chunks
  each), mm2 one psA + one psB per j, so the epilogue runs 1024-wide ops.
- The last j computes its B half first and splits the epilogue chain into
  512-wide halves so the tail after the final matmul is short.
"""

import numpy as np
import ml_dtypes
from contextlib import ExitStack

import concourse.bass as bass
import concourse.mybir as mybir
from concourse.tile import TileContext
from concourse.tile_rust import add_dep_helper
from concourse.bass_utils import run_bass_kernel_spmd

B, D, H = 2048, 4096, 1024
NCORES = 8
BS = B // NCORES            # 256 batch rows per core
KP1 = 16                    # mm1 streamed contract pairs: 16*256 = 4096 = D
KC2 = H // 128              # 8 contract chunks for matmul2
KP2 = KC2 // 2              # 4 DoubleRow pairs
NJ = D // 512               # 8 output column groups of 512
LEAKY_SLOPE = 0.01
LN_SQRT2 = 0.34657359027997264
LN2 = 0.6931471805599453
SX = 2.0**4                 # x fp8 scale
SW = 2.0**8                 # W1/W2 fp8 scale

F8 = mybir.dt.float8e4
F16 = mybir.dt.float16
BF16 = mybir.dt.bfloat16
F32 = mybir.dt.float32
AF = mybir.ActivationFunctionType
OP = mybir.AluOpType
DR = mybir.MatmulPerfMode.DoubleRow

NPF8 = ml_dtypes.float8_e4m3
NPBF16 = ml_dtypes.bfloat16


def split_multi_waits(nc):
    """This container's walrus accepts at most ONE sync-wait per instruction
    (setupSyncWait: 'Too many sync wait commands').  Split any instruction
    carrying N>1 waits into N-1 single-wait NoOps on the same engine placed
    immediately before it."""
    cnt = 0
    sync_info_cls = None
    for f in nc.m.functions:
        for bb in f.blocks:
            out = []
            changed = False
            for inst in bb.instructions:
                si = inst.sync_info
                waits = list(si.on_wait) if si and si.on_wait else []
                if len(waits) > 1:
                    if sync_info_cls is None:
                        sync_info_cls = type(si)
                    for w in waits[:-1]:
                        nop = mybir.InstNoOp(name=f"waitsplit_{cnt}", ins=[], outs=[])
                        cnt += 1
                        nop.engine = inst.engine
                        nop.sync_info = sync_info_cls(on_wait=[w], on_update=[])
                        out.append(nop)
                    si.on_wait = waits[-1:]
                    changed = True
                out.append(inst)
            if changed:
                bb.instructions = out
    return cnt


def _lean_drain_and_barrier(self, tick_clock, wait_clock):
    """Replacement for TileContext._drain_and_barrier: drain + ONE barrier,
    skipping the ~7us semaphore-clear butterfly.  The Bass preamble re-clears
    every kernel semaphore at the start of each execution, and no sibling
    TileContext follows this one, so the tail clear is redundant.  The
    multi-wait drain is split later by split_multi_waits."""
    import concourse.tile as tile_mod

    nc = self.nc
    drain_inst = nc.sync.drain()
    wait_clock.add_sem_waits(
        drain_inst.ins, tile_mod.ScopedClock({None: tick_clock.global_clock})
    )
    # No all_engine_barrier: the SP drain above waits on every semaphore's
    # final tick (all engines' last work and all DMA completions), so SP
    # retires last and execution end implies everything finished.
    popped = nc._tile_sem_poison_stack.pop()
    assert popped is self._sem_poison


def _build():
    nc = bass.Bass()
    orig_drain = TileContext._drain_and_barrier
    TileContext._drain_and_barrier = _lean_drain_and_barrier
    try:
        _build_body(nc)
    finally:
        TileContext._drain_and_barrier = orig_drain

    split_multi_waits(nc)
    return nc


def _build_body(nc):
    # All bulk tensors are FLAT per-partition so every DMA is one contiguous
    # multi-KB run per partition: the DGE generates descriptors at a fixed
    # ~80/us per queue, so sub-2KB descriptors cap a ring at ~160 GB/s
    # (measured: W1's 4D pattern starved mm1 at ~70-125 GB/s).
    xT = nc.dram_tensor("xT", [128, KP1 * 2 * BS], F8, kind="ExternalInput")
    w1 = nc.dram_tensor("w1", [128, KP1 * 2 * H], F8, kind="ExternalInput")
    w2 = nc.dram_tensor("w2", [128, NJ, KC2, 2, 512], F8, kind="ExternalInput")
    # rank-2 seed operands: row0 = (b1*2^12, ones), row1 = (W1[4096]*2^8, t*2^4)
    seedw = nc.dram_tensor("seedw", [2, H], F8, kind="ExternalInput")
    seedx = nc.dram_tensor("seedx", [2, BS], F8, kind="ExternalInput")
    # mun/out packed [p, j, h, col] -> flat [128, 8192]; host un/packs
    mun = nc.dram_tensor("mun", [128, NJ * 1024], BF16, kind="ExternalInput")
    outd = nc.dram_tensor("out", [128, NJ * 1024], F16, kind="ExternalOutput")

    with TileContext(nc) as tc, ExitStack() as ctx:
        const = ctx.enter_context(tc.tile_pool(name="const", bufs=1))
        xpool = ctx.enter_context(tc.tile_pool(name="xpool", bufs=1))
        w1pool = ctx.enter_context(tc.tile_pool(name="w1pool", bufs=4))
        hpool = ctx.enter_context(tc.tile_pool(name="hpool", bufs=1))
        w2pool = ctx.enter_context(tc.tile_pool(name="w2pool", bufs=6))
        eppool = ctx.enter_context(tc.tile_pool(name="eppool", bufs=4))
        outpool = ctx.enter_context(tc.tile_pool(name="outpool", bufs=3))
        pspool = ctx.enter_context(tc.tile_pool(name="pspool", bufs=4, space="PSUM"))

        # --- constants ---
        ones_row = const.tile([128, BS], F16, name="ones_row")
        nc.vector.memset(ones_row[:], 1.0)
        ones128 = const.tile([128, 128], F16, name="ones128")
        nc.vector.memset(ones128[:], 1.0)
        nln2_sb = const.tile([128, 1], F32, name="nln2_sb")
        nc.vector.memset(nln2_sb[:], -(LN_SQRT2 + 12.0 * LN2))
        scratch = const.tile([128, 1], F32, name="scratch")


        # short PE warm-up: dependency-free full-rank matmuls open the HAM
        # clock gate (needs ~3.5us of PE-busy; rank-2 seeds don't count)
        ps_warm = pspool.tile([128, 1024], F32, tag="ps", name="ps_warm")

        def warmup(n):
            for _ in range(n):
                nc.tensor.matmul(
                    ps_warm[:, :BS], ones128[:], ones_row[:], start=True, stop=True
                )

        warmup(22)

        # tiny rank-2 seed operand loads FIRST on the Scalar HWDGE ring
        # (the SWDGE ring takes ~3.5us to deliver its first byte, which
        # stalled the PE between warm-up and mm1 and jittered the HAM boost)
        seedw_sb = const.tile([2, H], F8, name="seedw_sb")
        nc.scalar.dma_start(out=seedw_sb[:], in_=seedw[:])
        seedx_sb = const.tile([2, BS], F8, name="seedx_sb")
        nc.scalar.dma_start(out=seedx_sb[:], in_=seedx[:])

        # --- x^T resident, pair-packed; Scalar HWDGE ring (the SWDGE ring
        # ramps far too slowly at kernel start), one flat tile, split so
        # mm1's first pairs don't wait for the whole 1 MB.
        XT_PARTS = [4, 6, 6]  # pairs per part; front-load small
        xt_flat = xpool.tile([128, KP1 * 2 * BS], F8, name="xt_flat")
        q0 = 0
        for npair in XT_PARTS:
            nc.scalar.dma_start(
                out=xt_flat[:, q0 * 512 : (q0 + npair) * 512],
                in_=xT[:, q0 * 512 : (q0 + npair) * 512],
            )
            q0 += npair
        assert q0 == KP1

        # preload the Lrelu ACT table (1.5us load, hidden under mm1)
        nc.scalar.activation(scratch[:], nln2_sb[:], AF.Lrelu, alpha=LEAKY_SLOPE)

        def xt_pair(q):
            return xt_flat[:, q * 512 : (q + 1) * 512].rearrange(
                "p (r b) -> p r b", r=2
            )

        # --- matmul1: h^T = W1^T @ x^T, H on partitions, fp8 DoubleRow.
        # PSUM: two [128,1024] double-bank tiles, 4 m-chunks (256 cols) each.
        ps1_t = [
            pspool.tile([128, 1024], F32, tag="ps", name=f"ps1_t{i}")
            for i in range(2)
        ]

        def ps1(m):
            return ps1_t[m // 4][:, (m % 4) * BS : (m % 4 + 1) * BS]

        # rank-2 seeds (b1 bias + t-row contribution) start the mm1
        # accumulation; a few more warm-up matmuls after them bridge the
        # HAM-invisible window (rank-2 work doesn't count as PE-busy).
        for m in range(KC2):
            nc.tensor.matmul(
                ps1(m),
                seedw_sb[:, m * 128 : (m + 1) * 128],
                seedx_sb[:],
                start=True,
                stop=False,
            )
        warmup(4)

        W1_PARTS = [1, 1, 2, 4, 4, 4]  # pairs per group, Sync ring, flat tiles
        PW = 2 * H  # elements per pair per partition
        mm1_last = {}
        q = 0
        for g, npair in enumerate(W1_PARTS):
            w1g = w1pool.tile(
                [128, max(W1_PARTS) * PW], F8, tag="w1t", name=f"w1g{g}"
            )
            nc.sync.dma_start(
                out=w1g[:, : npair * PW],
                in_=w1[:, q * PW : (q + npair) * PW],
            )
            for i in range(npair):
                rhs = xt_pair(q)
                w1p = w1g[:, i * PW : (i + 1) * PW].rearrange(
                    "p (r h) -> p r h", r=2
                )
                for m in range(KC2):
                    mm = nc.tensor.matmul(
                        ps1(m),
                        w1p[:, :, m * 128 : (m + 1) * 128],
                        rhs,
                        start=False,
                        stop=(q == KP1 - 1),
                        perf_mode=DR,
                    )
                mm1_last[q] = mm
                q += 1
        assert q == KP1

        # h fp8 at 2^4 scale, all 8 chunks in one [128, 8, 256] tile;
        # Lrelu(2^-8 * psum) = 2^4 * Lrelu(xW1 + b1): bias was seeded, so
        # TWO merged 1024-wide Lrelus cover all 8 chunks.
        h_all = hpool.tile([128, KC2, BS], F8, name="h_all")
        for i in range(2):
            nc.scalar.activation(
                h_all[:, 4 * i : 4 * (i + 1), :],
                ps1_t[i][:],
                AF.Lrelu,
                scale=2.0**-8,
                alpha=LEAKY_SLOPE,
            )
        # preload the Exp table while mm2's first matmuls run
        nc.scalar.activation(scratch[:], nln2_sb[:], AF.Exp)

        # --- matmul2 + fused epilogue ---
        w2ts, mu_js = {}, {}
        for j in range(NJ):
            w2t = w2pool.tile([128, KC2, 2, 512], F8, tag="w2", name=f"w2t{j}")
            pace = {0: 7, 1: 10, 2: 13, 3: 15}.get(j)
            dma = nc.gpsimd.dma_start(out=w2t[:], in_=w2[:, j])
            if pace is not None:
                add_dep_helper(dma.ins, mm1_last[pace].ins, True, "pace w2")
            w2ts[j] = w2t
            mu_j = eppool.tile([128, 1024], BF16, tag="mu", name=f"mu{j}")
            dma = nc.scalar.dma_start(
                out=mu_j[:], in_=mun[:, j * 1024 : (j + 1) * 1024]
            )
            if j == 0:
                add_dep_helper(dma.ins, mm1_last[13].ins, True, "pace mu")
            mu_js[j] = mu_j

        def emit_matmuls(j, b_first):
            w2t = w2ts[j]
            psA = pspool.tile([128, 1024], F32, tag="ps", name=f"psA{j}")
            psB = pspool.tile([128, 1024], F32, tag="ps", name=f"psB{j}")
            dst = {0: psA, 1: psB}
            for h in ((1, 0) if b_first else (0, 1)):
                for qq in range(KP2):
                    for bh in range(2):
                        nc.tensor.matmul(
                            dst[h][:, bh * 512 : (bh + 1) * 512],
                            h_all[:, 2 * qq : 2 * qq + 2, bh * 128 : (bh + 1) * 128],
                            w2t[:, 2 * qq : 2 * qq + 2, h, :],
                            start=(qq == 0),
                            stop=(qq == KP2 - 1),
                            perf_mode=DR,
                        )
            return psA, psB

        g2s = {}
        pend_erf = []

        def flush_erfs(js):
            for j in js:
                o2 = outpool.tile([128, 1024], F16, tag="o", name=f"O{j}")
                if j == NJ - 1:
                    # split the last chain across ACT/DVE/GpSimd halves so
                    # the tail after the final matmul is ~half as deep
                    for bh in range(2):
                        sl = slice(bh * 512, (bh + 1) * 512)
                        r2 = eppool.tile([128, 512], BF16, tag="R", name=f"R{j}_{bh}")
                        nc.scalar.activation(r2[:], g2s[j][:, sl], AF.Erf)
                        eng = nc.gpsimd if bh == 0 else nc.vector
                        eng.tensor_scalar(o2[:, sl], r2[:], 0.5, 0.5, OP.mult, OP.add)
                        nc.sync.dma_start(
                            out=outd[:, j * 1024 + bh * 512 : j * 1024 + (bh + 1) * 512],
                            in_=o2[:, sl],
                        )
                else:
                    r2 = eppool.tile([128, 1024], BF16, tag="Rw", name=f"R{j}")
                    nc.scalar.activation(r2[:], g2s[j][:], AF.Erf)
                    nc.gpsimd.tensor_scalar(o2[:], r2[:], 0.5, 0.5, OP.mult, OP.add)
                    nc.sync.dma_start(
                        out=outd[:, j * 1024 : (j + 1) * 1024], in_=o2[:]
                    )

        # Flush AFTER j4/j6 so each Erf batch sits behind the Exp whose PSUM
        # release the j+2 matmuls need — an Erf batch queued before that Exp
        # stalls the PE on psB slots.
        ERF_FLUSH_AFTER = {4, 6}  # Erf batches [4,2,2]

        for j in range(NJ):
            last = j == NJ - 1
            psA, psB = emit_matmuls(j, b_first=last)
            s2 = eppool.tile([128, 1024], BF16, tag="S", name=f"S{j}")
            g2 = eppool.tile([128, 1024], BF16, tag="G", name=f"G{j}")
            e2 = eppool.tile([128, 1024], BF16, tag="E", name=f"E{j}")
            if last:
                # split the chain into 512-wide halves on the DVE (GpSimd
                # cannot read PSUM and is ~2x slower on tensor_tensor) so
                # the bh=0 half's Erf/out can start one op earlier.
                for bh in range(2):
                    sl = slice(bh * 512, (bh + 1) * 512)
                    nc.vector.tensor_tensor(
                        s2[:, sl], psA[:, sl], mu_js[j][:, sl], OP.add
                    )
                nc.scalar.activation(
                    e2[:], psB[:], AF.Exp, bias=nln2_sb[:], scale=-(2.0**-12)
                )
                for bh in range(2):
                    sl = slice(bh * 512, (bh + 1) * 512)
                    nc.vector.tensor_tensor(g2[:, sl], s2[:, sl], e2[:, sl], OP.mult)
            else:
                nc.vector.tensor_tensor(s2[:], psA[:], mu_js[j][:], OP.add)
                nc.scalar.activation(
                    e2[:], psB[:], AF.Exp, bias=nln2_sb[:], scale=-(2.0**-12)
                )
                nc.vector.tensor_tensor(g2[:], s2[:], e2[:], OP.mult)
            g2s[j] = g2
            pend_erf.append(j)
            if j in ERF_FLUSH_AFTER:
                flush_erfs(pend_erf[:-1])
                del pend_erf[:-1]
        flush_erfs(pend_erf)


_NC = None
_last_in_maps = None


def kernel(mu, t, gamma, W1, b1, W2, b2):
    global _NC
    if _NC is None:
        _NC = _build()
    nc = _NC

    f16 = np.float16
    f32 = np.float32

    def q8(a, scale):
        return np.clip(np.asarray(a, f32) * scale, -240.0, 240.0).astype(NPF8)

    # x^T = mu^T * 2^4 fp8, pair-packed [128, (q, r, b)] flat; the t column
    # is a rank-2 seed (with b1)
    Xt = q8(mu, SX).T                      # (D, B)
    w1_np = np.ascontiguousarray(
        q8(W1[: D], SW).reshape(KP1, 2, 128, H).transpose(2, 0, 1, 3)
    ).reshape(128, KP1 * 2 * H)
    seedw_np = np.stack([q8(b1, SX * SW), q8(W1[D], SW)])  # [2, H]

    b2_64 = np.asarray(b2, np.float64)
    b2A, b2B = b2_64[:D], b2_64[D:]
    EB = np.exp(-b2B)  # fold b2 of the B half as a per-col factor on W2A
    W2f = np.asarray(W2, f32).astype(np.float64)
    W2q = np.concatenate([W2f[:, :D] * EB[None, :], W2f[:, D:]], axis=1)
    # W2 pack [p, j, k, half, col] = W2q[k*128+p, half*D + j*512 + col]
    w2_np = np.ascontiguousarray(
        q8(W2q, SW).reshape(KC2, 128, 2, NJ, 512).transpose(1, 3, 0, 2, 4)
    )

    g64 = np.asarray(gamma, dtype=np.float64)[:, 0]
    s64 = np.sqrt((1.0 - g64) / g64)
    qm_t = -1.0 / (g64 * s64)
    qa_t = 0.875 / s64
    # M = 2^12 * (mu*qm + qa + b2A) * EB, bf16  (absmax ~4e5, well in range)
    mun2 = (
        (
            np.asarray(mu, np.float64) * qm_t[:, None]
            + qa_t[:, None]
            + b2A[None, :]
        )
        * EB[None, :]
        * (SX * SW)
    ).astype(NPBF16)
    t8 = q8(t, SX)  # (B, 1)

    in_maps = []
    for c in range(NCORES):
        sl = slice(c * BS, (c + 1) * BS)
        in_maps.append(
            {
                "xT": np.ascontiguousarray(
                    Xt[:, sl].reshape(KP1, 2, 128, BS).transpose(2, 0, 1, 3)
                ).reshape(128, KP1 * 2 * BS),
                "w1": w1_np,
                "w2": w2_np,
                "seedw": seedw_np,
                "seedx": np.stack(
                    [np.ones(BS, dtype=NPF8), t8[sl, 0].astype(NPF8)]
                ),
                # [p, j, h, c] = M[h*128+p, j*512+c], flat [128, 8192]
                "mun": np.ascontiguousarray(
                    mun2[sl].reshape(2, 128, NJ, 512).transpose(1, 2, 0, 3)
                ).reshape(128, NJ * 1024),
            }
        )

    global _last_in_maps
    _last_in_maps = in_maps

    res = run_bass_kernel_spmd(nc, in_maps, core_ids=list(range(NCORES)))
    # out dram is [p, (j, h, c)]; unpack to [b, d] = [h*128+p, j*512+c]
    return np.concatenate(
        [
            r["out"]
            .reshape(128, NJ, 2, 512)
            .transpose(2, 0, 1, 3)
            .reshape(BS, D)
            .astype(np.float32)
            for r in res.results
        ],
        axis=0,
    )


# revision 51
# speedup vs baseline: 1.2780x; 1.0840x over previous
"""Trainium2 Bass kernel for nn_DiscretisedBNF (histogram binning MLP).

Math: the reference's per-bin CDF sum telescopes exactly (kl_{k+1} == kr_k
bit-identically, and cdf(kl_0) = cdf(kr_0) = 0 since those bounds are <= -1),
so

    sum_k [cdf(kr_k) - cdf(kl_k)] = cdf(kr_{K-1}) = 0.5*(1 + erf((0.875-mu_x)*inv))

with mu_x = mu/gamma - s*mu_eps, inv = 1/(sigma_x*sqrt(2)), sigma_x =
s*exp(ln_sigma_eps), s = sqrt((1-gamma)/gamma).  Rearranged for the chip,
with every per-column constant folded on the host:

    arg = (psA + M) * e              psA = 2^12 * (h @ W2A')     (PSUM)
    M   = 2^12*(mu*qm + qa + b2A)*EB   (host-precomputed, bf16)
    e   = exp(-2^-12*psB - ln(sqrt2) - 12ln2)                    (= E*2^-12)
    W2A'= W2A * EB,  EB = exp(-b2B)   (b2 of the B half folded as a
                                       multiplicative per-col factor)
    out = 0.5*erf(arg) + 0.5

Precision: both matmuls run in fp8 e4m3 with perf_mode=DoubleRow (2 fp8
weights/cell, ~2x PE rate, half the fp16 DMA bytes).  Scales keep everything
in e4m3's normal range (max +-240): x by 2^4, W1/W2 by 2^8.  The b1 bias and
the t-row of the concat([mu,t]) input are seeded into the mm1 PSUM
accumulation by a single rank-2 matmul per m-chunk (so mm1's streamed
contraction is exactly D=4096 = 16 DoubleRow pairs, and the Lrelu needs no
per-tile bias -> two merged 1024-wide Lrelus).  Epilogue intermediates are
bf16.  Measured end-to-end rel err: 1.0e-2 vs the 2e-2 gate.

Sharding: pure data parallel - batch dim (2048) split 256 rows per core;
weights replicated.  DoubleRow wants k-chunk PAIRS interleaved on the same
128 partitions (AP [128, 2, free]); the host packs accordingly.

Scheduling notes (from HW traces):
- Every bulk DMA is one contiguous multi-KB run per partition (flat 2D dram
  tensors + flat tiles, matmul views via AP rearrange): the DGE generates
  ~80 descriptors/us per queue, so small-descriptor patterns cap a ring at
  ~100-160 GB/s while 4-8KB runs reach full HBM bandwidth.
- The HAM clock gate needs a few us of PE-busy to open (warm-up burst) and
  closes again on PE idle >~2-3us; rank-2 seeds don't count as busy, so a
  few more warm-up matmuls follow them.
- The ACT engine reloads its function table on every Exp<->Erf switch
  (1.3-1.5us), so Erfs are batched ([4,2,2]) behind the Exps whose PSUM
  releases gate the next j-group's matmuls, with dummy activations
  preloading the Lrelu/Exp tables during idle windows.
- PSUM is managed as [128,1024] two-bank tiles: mm1 uses 2 (4 m-chunks
  each), mm2 one psA + one psB per j, so the epilogue runs 1024-wide ops.
- The last j computes its B half first and splits the epilogue chain into
  512-wide halves so the tail after the final matmul is short.
"""

import numpy as np
import ml_dtypes
from contextlib import ExitStack

import concourse.bass as bass
import concourse.mybir as mybir
from concourse.tile import TileContext
from concourse.tile_rust import add_dep_helper
from concourse.bass_utils import run_bass_kernel_spmd

B, D, H = 2048, 4096, 1024
NCORES = 8
BS = B // NCORES            # 256 batch rows per core
KP1 = 16                    # mm1 streamed contract pairs: 16*256 = 4096 = D
KC2 = H // 128              # 8 contract chunks for matmul2
KP2 = KC2 // 2              # 4 DoubleRow pairs
NJ = D // 512               # 8 output column groups of 512
LEAKY_SLOPE = 0.01
LN_SQRT2 = 0.34657359027997264
LN2 = 0.6931471805599453
SX = 2.0**4                 # x fp8 scale
SW = 2.0**8                 # W1/W2 fp8 scale

F8 = mybir.dt.float8e4
F16 = mybir.dt.float16
BF16 = mybir.dt.bfloat16
F32 = mybir.dt.float32
AF = mybir.ActivationFunctionType
OP = mybir.AluOpType
DR = mybir.MatmulPerfMode.DoubleRow

NPF8 = ml_dtypes.float8_e4m3
NPBF16 = ml_dtypes.bfloat16


def split_multi_waits(nc):
    """This container's walrus accepts at most ONE sync-wait per instruction
    (setupSyncWait: 'Too many sync wait commands').  Split any instruction
    carrying N>1 waits into N-1 single-wait NoOps on the same engine placed
    immediately before it."""
    cnt = 0
    sync_info_cls = None
    for f in nc.m.functions:
        for bb in f.blocks:
            out = []
            changed = False
            for inst in bb.instructions:
                si = inst.sync_info
                waits = list(si.on_wait) if si and si.on_wait else []
                if len(waits) > 1:
                    if sync_info_cls is None:
                        sync_info_cls = type(si)
                    for w in waits[:-1]:
                        nop = mybir.InstNoOp(name=f"waitsplit_{cnt}", ins=[], outs=[])
                        cnt += 1
                        nop.engine = inst.engine
                        nop.sync_info = sync_info_cls(on_wait=[w], on_update=[])
                        out.append(nop)
                    si.on_wait = waits[-1:]
                    changed = True
                out.append(inst)
            if changed:
                bb.instructions = out
    return cnt


def _lean_drain_and_barrier(self, tick_clock, wait_clock):
    """Replacement for TileContext._drain_and_barrier: drain + ONE barrier,
    skipping the ~7us semaphore-clear butterfly.  The Bass preamble re-clears
    every kernel semaphore at the start of each execution, and no sibling
    TileContext follows this one, so the tail clear is redundant.  The
    multi-wait drain is split later by split_multi_waits."""
    import concourse.tile as tile_mod

    nc = self.nc
    drain_inst = nc.sync.drain()
    wait_clock.add_sem_waits(
        drain_inst.ins, tile_mod.ScopedClock({None: tick_clock.global_clock})
    )
    # No all_engine_barrier: the SP drain above waits on every semaphore's
    # final tick (all engines' last work and all DMA completions), so SP
    # retires last and execution end implies everything finished.
    popped = nc._tile_sem_poison_stack.pop()
    assert popped is self._sem_poison


def _build():
    nc = bass.Bass()
    orig_drain = TileContext._drain_and_barrier
    TileContext._drain_and_barrier = _lean_drain_and_barrier
    try:
        _build_body(nc)
    finally:
        TileContext._drain_and_barrier = orig_drain

    split_multi_waits(nc)
    return nc


def _build_body(nc):
    # All bulk tensors are FLAT per-partition so every DMA is one contiguous
    # multi-KB run per partition: the DGE generates descriptors at a fixed
    # ~80/us per queue, so sub-2KB descriptors cap a ring at ~160 GB/s
    # (measured: W1's 4D pattern starved mm1 at ~70-125 GB/s).
    xT = nc.dram_tensor("xT", [128, KP1 * 2 * BS], F8, kind="ExternalInput")
    w1 = nc.dram_tensor("w1", [128, KP1 * 2 * H], F8, kind="ExternalInput")
    w2 = nc.dram_tensor("w2", [128, NJ, KC2, 2, 512], F8, kind="ExternalInput")
    # rank-2 seed operands: row0 = (b1*2^12, ones), row1 = (W1[4096]*2^8, t*2^4)
    seedw = nc.dram_tensor("seedw", [2, H], F8, kind="ExternalInput")
    seedx = nc.dram_tensor("seedx", [2, BS], F8, kind="ExternalInput")
    # mun/out packed [p, j, h, col] -> flat [128, 8192]; host un/packs
    mun = nc.dram_tensor("mun", [128, NJ * 1024], BF16, kind="ExternalInput")
    outd = nc.dram_tensor("out", [128, NJ * 1024], F16, kind="ExternalOutput")

    with TileContext(nc) as tc, ExitStack() as ctx:
        const = ctx.enter_context(tc.tile_pool(name="const", bufs=1))
        xpool = ctx.enter_context(tc.tile_pool(name="xpool", bufs=1))
        w1pool = ctx.enter_context(tc.tile_pool(name="w1pool", bufs=4))
        hpool = ctx.enter_context(tc.tile_pool(name="hpool", bufs=1))
        w2pool = ctx.enter_context(tc.tile_pool(name="w2pool", bufs=6))
        eppool = ctx.enter_context(tc.tile_pool(name="eppool", bufs=4))
        outpool = ctx.enter_context(tc.tile_pool(name="outpool", bufs=3))
        pspool = ctx.enter_context(tc.tile_pool(name="pspool", bufs=4, space="PSUM"))

        # --- constants ---
        ones_row = const.tile([128, BS], F16, name="ones_row")
        nc.vector.memset(ones_row[:], 1.0)
        ones128 = const.tile([128, 128], F16, name="ones128")
        nc.vector.memset(ones128[:], 1.0)
        nln2_sb = const.tile([128, 1], F32, name="nln2_sb")
        nc.vector.memset(nln2_sb[:], -(LN_SQRT2 + 12.0 * LN2))
        scratch = const.tile([128, 1], F32, name="scratch")


        # short PE warm-up: dependency-free full-rank matmuls open the HAM
        # clock gate (needs ~3.5us of PE-busy; rank-2 seeds don't count)
        ps_warm = pspool.tile([128, 1024], F32, tag="ps", name="ps_warm")

        def warmup(n):
            for _ in range(n):
                nc.tensor.matmul(
                    ps_warm[:, :BS], ones128[:], ones_row[:], start=True, stop=True
                )

        warmup(22)

        # tiny rank-2 seed operand loads FIRST on the Scalar HWDGE ring
        # (the SWDGE ring takes ~3.5us to deliver its first byte, which
        # stalled the PE between warm-up and mm1 and jittered the HAM boost)
        seedw_sb = const.tile([2, H], F8, name="seedw_sb")
        nc.scalar.dma_start(out=seedw_sb[:], in_=seedw[:])
        seedx_sb = const.tile([2, BS], F8, name="seedx_sb")
        nc.scalar.dma_start(out=seedx_sb[:], in_=seedx[:])

        # --- x^T resident, pair-packed; Scalar HWDGE ring (the SWDGE ring
        # ramps far too slowly at kernel start), one flat tile, split so
        # mm1's first pairs don't wait for the whole 1 MB.
        XT_PARTS = [4, 6, 6]  # pairs per part; front-load small
        xt_flat = xpool.tile([128, KP1 * 2 * BS], F8, name="xt_flat")
        q0 = 0
        for npair in XT_PARTS:
            nc.scalar.dma_start(
                out=xt_flat[:, q0 * 512 : (q0 + npair) * 512],
                in_=xT[:, q0 * 512 : (q0 + npair) * 512],
            )
            q0 += npair
        assert q0 == KP1

        # preload the Lrelu ACT table (1.5us load, hidden under mm1)
        nc.scalar.activation(scratch[:], nln2_sb[:], AF.Lrelu, alpha=LEAKY_SLOPE)

        def xt_pair(q):
            return xt_flat[:, q * 512 : (q + 1) * 512].rearrange(
                "p (r b) -> p r b", r=2
            )

        # --- matmul1: h^T = W1^T @ x^T, H on partitions, fp8 DoubleRow.
        # PSUM: two [128,1024] double-bank tiles, 4 m-chunks (256 cols) each.
        ps1_t = [
            pspool.tile([128, 1024], F32, tag="ps", name=f"ps1_t{i}")
            for i in range(2)
        ]

        def ps1(m):
            return ps1_t[m // 4][:, (m % 4) * BS : (m % 4 + 1) * BS]

        # rank-2 seeds (b1 bias + t-row contribution) start the mm1
        # accumulation; a few more warm-up matmuls after them bridge the
        # HAM-invisible window (rank-2 work doesn't count as PE-busy).
        for m in range(KC2):
            nc.tensor.matmul(
                ps1(m),
                seedw_sb[:, m * 128 : (m + 1) * 128],
                seedx_sb[:],
                start=True,
                stop=False,
            )
        warmup(4)

        W1_PARTS = [1, 1, 2, 4, 4, 4]  # pairs per group, Sync ring, flat tiles
        PW = 2 * H  # elements per pair per partition
        mm1_last = {}
        q = 0
        for g, npair in enumerate(W1_PARTS):
            w1g = w1pool.tile(
                [128, max(W1_PARTS) * PW], F8, tag="w1t", name=f"w1g{g}"
            )
            nc.sync.dma_start(
                out=w1g[:, : npair * PW],
                in_=w1[:, q * PW : (q + npair) * PW],
            )
            for i in range(npair):
                rhs = xt_pair(q)
                w1p = w1g[:, i * PW : (i + 1) * PW].rearrange(
                    "p (r h) -> p r h", r=2
                )
                for m in range(KC2):
                    mm = nc.tensor.matmul(
                        ps1(m),
                        w1p[:, :, m * 128 : (m + 1) * 128],
                        rhs,
                        start=False,
                        stop=(q == KP1 - 1),
                        perf_mode=DR,
                    )
                mm1_last[q] = mm
                q += 1
        assert q == KP1

        # h fp8 at 2^4 scale, all 8 chunks in one [128, 8, 256] tile;
        # Lrelu(2^-8 * psum) = 2^4 * Lrelu(xW1 + b1): bias was seeded, so
        # TWO merged 1024-wide Lrelus cover all 8 chunks.
        h_all = hpool.tile([128, KC2, BS], F8, name="h_all")
        for i in range(2):
            nc.scalar.activation(
                h_all[:, 4 * i : 4 * (i + 1), :],
                ps1_t[i][:],
                AF.Lrelu,
                scale=2.0**-8,
                alpha=LEAKY_SLOPE,
            )
        # preload the Exp table while mm2's first matmuls run
        nc.scalar.activation(scratch[:], nln2_sb[:], AF.Exp)

        # --- matmul2 + fused epilogue ---
        w2ts, mu_js = {}, {}
        for j in range(NJ):
            w2t = w2pool.tile([128, KC2, 2, 512], F8, tag="w2", name=f"w2t{j}")
            pace = {0: 7, 1: 10, 2: 13, 3: 15}.get(j)
            dma = nc.gpsimd.dma_start(out=w2t[:], in_=w2[:, j])
            if pace is not None:
                add_dep_helper(dma.ins, mm1_last[pace].ins, True, "pace w2")
            w2ts[j] = w2t
            mu_j = eppool.tile([128, 1024], BF16, tag="mu", name=f"mu{j}")
            dma = nc.scalar.dma_start(
                out=mu_j[:], in_=mun[:, j * 1024 : (j + 1) * 1024]
            )
            if j == 0:
                add_dep_helper(dma.ins, mm1_last[13].ins, True, "pace mu")
            mu_js[j] = mu_j

        def emit_matmuls(j, b_first):
            w2t = w2ts[j]
            psA = pspool.tile([128, 1024], F32, tag="ps", name=f"psA{j}")
            psB = pspool.tile([128, 1024], F32, tag="ps", name=f"psB{j}")
            dst = {0: psA, 1: psB}
            for h in ((1, 0) if b_first else (0, 1)):
                for qq in range(KP2):
                    for bh in range(2):
                        nc.tensor.matmul(
                            dst[h][:, bh * 512 : (bh + 1) * 512],
                            h_all[:, 2 * qq : 2 * qq + 2, bh * 128 : (bh + 1) * 128],
                            w2t[:, 2 * qq : 2 * qq + 2, h, :],
                            start=(qq == 0),
                            stop=(qq == KP2 - 1),
                            perf_mode=DR,
                        )
            return psA, psB

        g2s = {}
        pend_erf = []

        def flush_erfs(js):
            for j in js:
                o2 = outpool.tile([128, 1024], F16, tag="o", name=f"O{j}")
                if j == NJ - 1:
                    # split the last chain across ACT/DVE/GpSimd halves so
                    # the tail after the final matmul is ~half as deep
                    for bh in range(2):
                        sl = slice(bh * 512, (bh + 1) * 512)
                        r2 = eppool.tile([128, 512], BF16, tag="R", name=f"R{j}_{bh}")
                        nc.scalar.activation(r2[:], g2s[j][:, sl], AF.Erf)
                        eng = nc.gpsimd if bh == 0 else nc.vector
                        eng.tensor_scalar(o2[:, sl], r2[:], 0.5, 0.5, OP.mult, OP.add)
                        nc.sync.dma_start(
                            out=outd[:, j * 1024 + bh * 512 : j * 1024 + (bh + 1) * 512],
                            in_=o2[:, sl],
                        )
                else:
                    r2 = eppool.tile([128, 1024], BF16, tag="Rw", name=f"R{j}")
                    nc.scalar.activation(r2[:], g2s[j][:], AF.Erf)
                    nc.gpsimd.tensor_scalar(o2[:], r2[:], 0.5, 0.5, OP.mult, OP.add)
                    nc.sync.dma_start(
                        out=outd[:, j * 1024 : (j + 1) * 1024], in_=o2[:]
                    )

        # Flush AFTER j4/j6 so each Erf batch sits behind the Exp whose PSUM
        # release the j+2 matmuls need — an Erf batch queued before that Exp
        # stalls the PE on psB slots.
        ERF_FLUSH_AFTER = {4, 6}  # Erf batches [4,2,2]

        for j in range(NJ):
            last = j == NJ - 1
            psA, psB = emit_matmuls(j, b_first=last)
            s2 = eppool.tile([128, 1024], BF16, tag="S", name=f"S{j}")
            g2 = eppool.tile([128, 1024], BF16, tag="G", name=f"G{j}")
            e2 = eppool.tile([128, 1024], BF16, tag="E", name=f"E{j}")
            if last:
                # split the chain into 512-wide halves on the DVE (GpSimd
                # cannot read PSUM and is ~2x slower on tensor_tensor) so
                # the bh=0 half's Erf/out can start one op earlier.
                for bh in range(2):
                    sl = slice(bh * 512, (bh + 1) * 512)
                    nc.vector.tensor_tensor(
                        s2[:, sl], psA[:, sl], mu_js[j][:, sl], OP.add
                    )
                nc.scalar.activation(
                    e2[:], psB[:], AF.Exp, bias=nln2_sb[:], scale=-(2.0**-12)
                )
                for bh in range(2):
                    sl = slice(bh * 512, (bh + 1) * 512)
                    nc.vector.tensor_tensor(g2[:, sl], s2[:, sl], e2[:, sl], OP.mult)
            else:
                nc.vector.tensor_tensor(s2[:], psA[:], mu_js[j][:], OP.add)
                nc.scalar.activation(
                    e2[:], psB[:], AF.Exp, bias=nln2_sb[:], scale=-(2.0**-12)
                )
                nc.vector.tensor_tensor(g2[:], s2[:], e2[:], OP.mult)
            g2s[j] = g2
            pend_erf.append(j)
            if j in ERF_FLUSH_AFTER:
                flush_erfs(pend_erf[:-1])
                del pend_erf[:-1]
        flush_erfs(pend_erf)


_NC = None
_last_in_maps = None


def kernel(mu, t, gamma, W1, b1, W2, b2):
    global _NC
    if _NC is None:
        _NC = _build()
    nc = _NC

    f16 = np.float16
    f32 = np.float32

    def q8(a, scale):
        return np.clip(np.asarray(a, f32) * scale, -240.0, 240.0).astype(NPF8)

    # x^T = mu^T * 2^4 fp8, pair-packed [128, (q, r, b)] flat; the t column
    # is a rank-2 seed (with b1)
    Xt = q8(mu, SX).T                      # (D, B)
    w1_np = np.ascontiguousarray(
        q8(W1[: D], SW).reshape(KP1, 2, 128, H).transpose(2, 0, 1, 3)
    ).reshape(128, KP1 * 2 * H)
    seedw_np = np.stack([q8(b1, SX * SW), q8(W1[D], SW)])  # [2, H]

    b2_64 = np.asarray(b2, np.float64)
    b2A, b2B = b2_64[:D], b2_64[D:]
    EB = np.exp(-b2B)  # fold b2 of the B half as a per-col factor on W2A
    W2f = np.asarray(W2, f32).astype(np.float64)
    W2q = np.concatenate([W2f[:, :D] * EB[None, :], W2f[:, D:]], axis=1)
    # W2 pack [p, j, k, half, col] = W2q[k*128+p, half*D + j*512 + col]
    w2_np = np.ascontiguousarray(
        q8(W2q, SW).reshape(KC2, 128, 2, NJ, 512).transpose(1, 3, 0, 2, 4)
    )

    g64 = np.asarray(gamma, dtype=np.float64)[:, 0]
    s64 = np.sqrt((1.0 - g64) / g64)
    qm_t = -1.0 / (g64 * s64)
    qa_t = 0.875 / s64
    # M = 2^12 * (mu*qm + qa + b2A) * EB, bf16  (absmax ~4e5, well in range)
    mun2 = (
        (
            np.asarray(mu, np.float64) * qm_t[:, None]
            + qa_t[:, None]
            + b2A[None, :]
        )
        * EB[None, :]
        * (SX * SW)
    ).astype(NPBF16)
    t8 = q8(t, SX)  # (B, 1)

    in_maps = []
    for c in range(NCORES):
        sl = slice(c * BS, (c + 1) * BS)
        in_maps.append(
            {
                "xT": np.ascontiguousarray(
                    Xt[:, sl].reshape(KP1, 2, 128, BS).transpose(2, 0, 1, 3)
                ).reshape(128, KP1 * 2 * BS),
                "w1": w1_np,
                "w2": w2_np,
                "seedw": seedw_np,
                "seedx": np.stack(
                    [np.ones(BS, dtype=NPF8), t8[sl, 0].astype(NPF8)]
                ),
                # [p, j, h, c] = M[h*128+p, j*512+c], flat [128, 8192]
                "mun": np.ascontiguousarray(
                    mun2[sl].reshape(2, 128, NJ, 512).transpose(1, 2, 0, 3)
                ).reshape(128, NJ * 1024),
            }
        )

    global _last_in_maps
    _last_in_maps = in_maps

    res = run_bass_kernel_spmd(nc, in_maps, core_ids=list(range(NCORES)))
    # out dram is [p, (j, h, c)]; unpack to [b, d] = [h*128+p, j*512+c]
    return np.concatenate(
        [
            r["out"]
            .reshape(128, NJ, 2, 512)
            .transpose(2, 0, 1, 3)
            .reshape(BS, D)
            .astype(np.float32)
            for r in res.results
        ],
        axis=0,
    )
